# revision 75
# baseline (speedup 1.0000x reference)
"""Trainium2 Bass kernel for nn_MultiHeadAttention_67018669687091.

Problem: MHA with B=2, S=2048, E=1024, H=16, D=64, causal, fp32.
The reference reshapes (B,S,E)->(B,H,S,D) WITHOUT transpose, so head h of
batch b is the contiguous 128-row x-block rows [h*128,(h+1)*128) viewed as a
(2048, 64) pseudo-sequence: position 16*s+j <- (row s, channel 64j+d).

Sharding: 8 cores; core c owns batch b=c//4 and head-quad qd=c%4 (4 heads).
Each core computes the qkv projection for its 4 blocks, per-head causal
attention in the transposed domain (scores with key on partitions, softmax
along the free dim via an augmented ones-row of V and late normalization),
and a row-parallel slice of the output projection. Host sums the 4 partials
per batch and adds bout.

Performance notes (per-core, PE-column-bound; the PE clock is power-governed
so matmul column count is the controlling resource):
 - QT/KT/VT are stored chunk-major permuted (col = 128*kc + 8*jb + s8) so
   the qkv->per-head scatter copies are contiguous-innermost on the DVE
   (4.6us -> ~1.2us per copy); all causal chunk slicing is unchanged, the
   host-built mask handles the within-chunk order, and the permutation is
   undone for free in the con write.
 - Inputs are host-prepacked tile-contiguous; x/bias issue from the Scalar
   HWDGE queue in parallel with weights on Sync (dma_start issue time, not
   bandwidth, gates the prologue). The partial output is written tile-major.
 - Softmax denominator reciprocal: partition-spread via a DRAM round trip,
   emitted in deferred slices across later flush points so no in-order
   engine queue waits on a DMA hop; the con multiply runs on GpSimd.
 - Attention-phase PSUM pools close before the output projection so it gets
   a 5-deep pool (with 2 bufs each tile's matmuls serialize behind the
   previous tile's staging copy).
 - fp8 DoubleRow PV (pv_fp8=True) works but is a net loss: the power
   governor tracks MAC-rate, which DoubleRow does not reduce, and exp
   overflows e4m3 without per-head max subtraction (NaN). Kept for
   reference, off by default.

Matmul operand dtype is selectable: bfloat16 (fastest), float32r, float32.
"""
import numpy as np
from contextlib import ExitStack

import concourse.bass as bass
import concourse.bacc as bacc
import concourse.mybir as mybir
import concourse.tile as tile
from concourse.masks import make_identity
from concourse.bass_utils import run_bass_kernel_spmd

E = 1024
H = 16
D = 64
B = 2
S = 2048
HPC = 4          # heads per core
SL = HPC * 128   # x columns per core (512)

F32 = mybir.dt.float32
F32R = mybir.dt.float32r
BF16 = mybir.dt.bfloat16
FP8 = mybir.dt.float8e4
EXP = mybir.ActivationFunctionType.Exp
DR = mybir.MatmulPerfMode.DoubleRow


def _pieces(lo, hi, bank=512):
    """Split [lo, hi) at multiples of `bank` (PSUM bank boundaries)."""
    out = []
    while lo < hi:
        nxt = min(hi, (lo // bank + 1) * bank)
        out.append((lo, nxt))
        lo = nxt
    return out


def build_program(mm_dt=BF16, pv_fp8=False):
    """One SPMD program; per-core data comes via in_maps."""
    nc = bacc.Bacc("TRN2", target_bir_lowering=False)
    DT = mm_dt
    # dtype of the qkv tiles / transposes: must pair legally with identity
    QKVDT = DT if DT == BF16 else F32

    # Inputs host-prepacked so that every load is a single clean 2D DMA with
    # contiguous per-partition lines (cheap Sync-issue descriptors):
    #   xq[hl*128+p, ec*128+s]      = x[b].T[ec*128+p, hl*128+s]
    #   wq[(t*8+ec)*128+p, c]       = Wqkv.T[ec*128+p, t*1024+c]
    #   woutq[p, hf*E+j]            = Wout.T[qd*256+hf*128+p, j]
    xq = nc.dram_tensor("xq", [HPC * 128, 1024], DT, kind="ExternalInput").ap()
    wq = nc.dram_tensor("wq", [24 * 128, 1024], DT, kind="ExternalInput").ap()
    bqkv = nc.dram_tensor("bqkv", [1, 3 * E], F32, kind="ExternalInput").ap()
    woutq = nc.dram_tensor("woutq", [128, 2 * E], DT, kind="ExternalInput").ap()
    maskd = nc.dram_tensor(
        "maskp", [128, 128], F32 if pv_fp8 else BF16, kind="ExternalInput").ap()
    # tile-major output: row block (sb*8+jc) holds the [128, 512] tile for
    # out channels jc*128.. and positions sb*512.. — every DMA write lands
    # fully contiguous in DRAM; the host assembles the final layout.
    outT = nc.dram_tensor("partialT", [32 * 128, 512], BF16,
                          kind="ExternalOutput").ap()

    with tile.TileContext(nc) as tc, ExitStack() as ctx:
        singles = ctx.enter_context(tc.tile_pool(name="singles", bufs=1))
        wpool = ctx.enter_context(tc.tile_pool(name="wpool", bufs=2))
        qkvpool = ctx.enter_context(tc.tile_pool(name="qkvpool", bufs=1))
        headpool = ctx.enter_context(tc.tile_pool(name="headpool", bufs=2))
        ppool = ctx.enter_context(tc.tile_pool(name="ppool", bufs=4))
        cpool = ctx.enter_context(tc.tile_pool(name="cpool", bufs=1))
        stpool = ctx.enter_context(tc.tile_pool(name="stpool", bufs=3))
        small = ctx.enter_context(tc.tile_pool(name="small", bufs=3))
        dpool = ctx.enter_context(tc.tile_pool(name="dpool", bufs=2, space="DRAM"))
        mmps = ctx.enter_context(tc.tile_pool(name="mmps", bufs=2, space="PSUM"))
        # attention-phase PSUM pools live in their own scope: they close
        # before the output projection so their 6 banks can be reused for a
        # deeply-buffered out-proj pool (bufs=2 there serializes each tile's
        # matmuls behind the previous tile's staging copy).
        attn_ctx = ExitStack()
        sps = attn_ctx.enter_context(tc.tile_pool(name="sps", bufs=2, space="PSUM"))
        ops = attn_ctx.enter_context(tc.tile_pool(name="ops", bufs=1, space="PSUM"))

        ident = singles.tile([128, 128], QKVDT, tag="ident")
        make_identity(nc, ident)

        # Diagonal-chunk causal mask, host-computed for the permuted key/query
        # order (idx = 8*jb + s8 -> pos 16*s8 + jb). pv_fp8: additive 0/-1e9
        # f32 mask applied to scores pre-exp. bf16: multiplicative 0/1 on P.
        # (loaded after the projection emission — not startup-critical)
        mask01 = singles.tile([128, 128], F32 if pv_fp8 else BF16, tag="mask01")

        # Startup-critical loads only: x tiles and the qkv bias. wout and the
        # mask are deferred until after the projection loop — every dma_start
        # costs ~0.65us of serial issue time on the Sync queue, so front
        # issue-count is what sets when the first matmul can run.
        # x + bias issue from the (startup-idle) Scalar queue, weights from
        # Sync — the two HWDGE issuers run in parallel, halving the serial
        # issue time in front of the first projection matmul.
        xts = []
        for hl in range(HPC):
            xt = singles.tile([128, 8, 128], DT, tag=f"xt{hl}")
            nc.scalar.dma_start(out=xt, in_=xq[hl * 128:(hl + 1) * 128, :])
            xts.append(xt)

        bias_sb = singles.tile([128, 3 * E], F32, tag="bias")
        bq_bcast = bass.AP(
            tensor=bqkv.tensor, offset=bqkv.offset,
            ap=[[0, 128]] + [list(d) for d in bqkv.ap[1:]],
        )
        nc.scalar.dma_start(out=bias_sb, in_=bq_bcast)

        # ---- QKV projection: qkv[hl] = x_blk @ WqkvT + bqkv  (128, 3072)
        qkvs = [
            qkvpool.tile([128, 3 * E], QKVDT, tag=f"qkv{hl}", name=f"qkv{hl}")
            for hl in range(HPC)
        ]

        # ---- per-head attention prep machinery. qkv columns of tensor t
        # (q/k/v) are exactly proj nb-blocks (2t, 2t+1), so each tensor's
        # transposes + scatter copies are emitted right after its two proj
        # blocks: the DVE scatter stream for heads 0/1 runs during the
        # projection (where DVE is otherwise idle) instead of after it.
        conA = cpool.tile([128, S], DT, tag="conA")
        conB = cpool.tile([128, S], DT, tag="conB")
        preps = {}

        def head_tiles(hl):
            if hl not in preps:
                # QT/KT zero-padded to 128 partitions: K=128 full-array S
                # matmuls stream at 216ns (K=64 serializes LDWEIGHTS).
                QT = headpool.tile([128, S], DT, tag="QT", name=f"QT{hl}")
                KT = headpool.tile([128, S], DT, tag="KT", name=f"KT{hl}")
                VT = headpool.tile([65, S], QKVDT, tag="VT", name=f"VT{hl}")
                # pv_fp8: inner dim padded to 80 — DoubleRow LDWEIGHTS requires
                # the K-pair stride to be a multiple of 16.
                Vc = headpool.tile([128, 16, 80 if pv_fp8 else 65],
                                   FP8 if pv_fp8 else DT,
                                   tag="Vc", name=f"Vc{hl}")
                if hl < 2:
                    # pads/ones are written once per pool slot (bufs=2, slots
                    # alternate hl%2); heads 2/3 reuse them — scatters only
                    # touch rows 0:64, so the constant rows persist and the
                    # per-head memset (plus its WAR serialization in the prep
                    # chain) is skipped.
                    nc.gpsimd.memset(QT[64:128, :], 0.0)
                    nc.gpsimd.memset(KT[64:128, :], 0.0)
                    nc.gpsimd.memset(VT[64:65, :], 1.0)
                preps[hl] = [QT, KT, VT, Vc]
            return preps[hl]

        def emit_tensor_prep(hl, t):
            # QT/KT/VT hold the head's (d, pseudo-seq) slab chunk-major:
            # column 128*kc + 8*jb + s8, for pseudo-position 16*(8*kc+s8)+jb.
            # The permutation is local to each 128-chunk, so all causal
            # chunk/piece slicing is unchanged; the scatter copy below gets a
            # contiguous innermost run (fast on DVE) instead of a stride-16
            # element scatter (4.6us -> ~1.2us per copy). Scores/P/outp/stg
            # inherit the within-chunk column order; it is undone in the con
            # write, and the host-built mask01 accounts for it.
            dest = head_tiles(hl)[t]
            tpb = mmps.tile([128, 1024], QKVDT, tag="mm", space="PSUM",
                            name=f"tpb{hl}_{t}")
            for cc in range(8):
                nc.tensor.transpose(
                    tpb[:, cc * 128:(cc + 1) * 128],
                    qkvs[hl][:, t * 1024 + cc * 128: t * 1024 + (cc + 1) * 128],
                    ident,
                )
            t3 = tpb.rearrange("p (cc s) -> p cc s", cc=8)
            for jp in range(2):
                csrc = t3[64 * jp:64 * jp + 64, :, :].rearrange(
                    "d cc (kc sk) -> d cc kc sk", kc=16)
                dd = dest[0:64].rearrange(
                    "d (kc cc two sk) -> d two cc kc sk",
                    kc=16, cc=8, two=2)[:, jp, :, :, :]
                nc.vector.tensor_copy(dd, csrc)

        def emit_vc_prep(hl):
            QT, KT, VT, Vc = head_tiles(hl)
            for kc in range(16):
                tp = mmps.tile([128, 512], QKVDT, tag="mm", space="PSUM",
                               name=f"tpv{hl}_{kc}")
                nc.tensor.transpose(
                    tp[:, 0:65], VT[:, kc * 128:(kc + 1) * 128], ident[0:65, 0:65])
                nc.vector.tensor_copy(Vc[:, kc, 0:65], tp[:, 0:65])

        def emit_prep(hl):
            for t in range(3):
                emit_tensor_prep(hl, t)
            emit_vc_prep(hl)

        # ---- QKV projection. Weights come in as one [128, 1024] DMA per
        # (t, ec) covering both nb blocks of tensor t: halves the number of
        # Sync-issued dma_starts on the startup-critical path.
        for t in range(3):
            wts = []
            for ec in range(8):
                wtc = wpool.tile([128, 1024], DT, tag=f"wt{ec}", name=f"wt{t}_{ec}")
                r = (t * 8 + ec) * 128
                # t=0 chunks gate the first matmul: split their issues
                # across both HWDGE queues
                eng = nc.scalar if (t == 0 and ec >= 4) else nc.sync
                eng.dma_start(out=wtc, in_=wq[r:r + 128, :])
                wts.append(wtc)
            for nb in (2 * t, 2 * t + 1):
                h = (nb % 2) * 512
                for hl in range(HPC):
                    ps = mmps.tile([128, 512], F32, tag="mm")
                    for ec in range(8):
                        nc.tensor.matmul(
                            ps, lhsT=xts[hl][:, ec, :], rhs=wts[ec][:, h:h + 512],
                            start=(ec == 0), stop=(ec == 7),
                        )
                    nc.vector.tensor_add(
                        qkvs[hl][:, nb * 512:(nb + 1) * 512], ps,
                        bias_sb[:, nb * 512:(nb + 1) * 512],
                    )

        # deferred non-critical input loads (used from attention onward)
        nc.sync.dma_start(out=mask01, in_=maskd)
        wout_sb = singles.tile([128, 2, E], DT, tag="wout")
        nc.sync.dma_start(out=wout_sb, in_=woutq.rearrange("p (hf j) -> p hf j", hf=2))

        def emit_scores(St, kc, qstart, qlen, KT, QT):
            for (a, b) in _pieces(0, qlen):
                nc.tensor.matmul(
                    St[:, a:b],
                    lhsT=KT[:, kc * 128:(kc + 1) * 128],
                    rhs=QT[:, qstart + a: qstart + b],
                    start=True, stop=True,
                )
            if kc * 128 == qstart and pv_fp8:
                # additive -1e9 mask on the diagonal chunk, pre-exp
                nc.vector.tensor_add(St[:, 0:128], St[:, 0:128], mask01)

        def emit_attention_fp8(hl, qh, outp, KT, QT, Vc):
            # Key chunks processed in pairs (2c, 2c+1); P stored as fp8 planes
            # and PV runs as K=256 DoubleRow matmuls (half the PE columns).
            # The even chunk's extra 128 queries (diagonal sliver) get a
            # separate plain-fp8 matmul.
            npairs = 4 * (qh + 1)
            relPs = [max(128 * (2 * c + 1) - 1024 * qh, 0) for c in range(npairs)]
            bank_last = {
                bk: max(c for c in range(npairs) if relPs[c] < 512 * (bk + 1))
                for bk in range(2)
            }
            for c in range(npairs):
                ke, ko = 2 * c, 2 * c + 1
                qs_e = max(ke * 128, qh * 1024)
                qs_o = max(ko * 128, qh * 1024)
                sliver = qs_o - qs_e  # 0 or 128
                qlen_e = (qh + 1) * 1024 - qs_e
                qlen_o = (qh + 1) * 1024 - qs_o
                P8t = ppool.tile([128, 2, 1024], FP8, tag="P",
                                 name=f"P{hl}_{qh}_{c}")
                for (kk, qs, qlen, pl) in ((ke, qs_e, qlen_e, 0),
                                           (ko, qs_o, qlen_o, 1)):
                    St = sps.tile([128, 1024], F32, tag="S", space="PSUM",
                                  name=f"St{hl}_{qh}_{kk}")
                    emit_scores(St, kk, qs, qlen, KT, QT)
                    nc.scalar.activation(P8t[:, pl, 0:qlen], St[:, 0:qlen],
                                         EXP, scale=0.125)
                relP = relPs[c]
                if sliver:
                    a = relP - 128
                    nc.tensor.matmul(
                        outp[a // 512][:, a % 512:a % 512 + 128],
                        lhsT=Vc[:, ke, 0:65],
                        rhs=P8t[:, 0, 0:128],
                        start=(c == 0), stop=False,
                    )
                for (a, b) in _pieces(relP, relP + qlen_o):
                    rhs = bass.AP(
                        tensor=P8t.tensor,
                        offset=P8t.offset + sliver + (a - relP),
                        ap=[list(P8t.ap[0]), [1024 - sliver, 2], [1, b - a]],
                    )
                    nc.tensor.matmul(
                        outp[a // 512][:, a % 512:a % 512 + b - a],
                        lhsT=Vc[:, ke:ke + 2, 0:65],
                        rhs=rhs,
                        start=(c == 0), stop=(c == bank_last[a // 512]),
                        perf_mode=DR,
                    )

        def emit_attention_bf16(hl, qh, outp, KT, QT, Vc):
            # Key chunks whose query lengths sum to 1024 share one St tile
            # and ONE exp: the causal staircase pairs up exactly (896+128,
            # 768+256, 640+384, ...), cutting ACTIVATE count from 24 to 18
            # per head (~300ns fixed overhead each, and fewer exp-latency
            # ping-pong points for the PE).
            items = []
            for kc in range(8 * (qh + 1)):
                qstart = max(kc * 128, qh * 1024)
                items.append((kc, qstart, (qh + 1) * 1024 - qstart))
            full = [[it] for it in items if it[2] >= 1024]
            rest = sorted((it for it in items if it[2] < 1024),
                          key=lambda it: -it[2])
            groups = list(full)
            i, j = 0, len(rest) - 1
            while i <= j:
                if i < j and rest[i][2] + rest[j][2] <= 1024:
                    groups.append([rest[i], rest[j]])
                    i, j = i + 1, j - 1
                else:
                    groups.append([rest[i]])
                    i += 1
            groups.sort(key=lambda g: min(it[0] for it in g))
            # per-PSUM-bank last writer under the actual emission order
            bank_last = {}
            for g in groups:
                for (kc, qstart, qlen) in g:
                    rel = qstart - qh * 1024
                    for (a, b) in _pieces(rel, rel + qlen):
                        bank_last[a // 512] = kc
            for g in groups:
                St = sps.tile([128, 1024], F32, tag="S", space="PSUM",
                              name=f"St{hl}_{qh}_{g[0][0]}")
                off, offs = 0, []
                for (kc, qstart, qlen) in g:
                    for (a, b) in _pieces(off, off + qlen):
                        nc.tensor.matmul(
                            St[:, a:b],
                            lhsT=KT[:, kc * 128:(kc + 1) * 128],
                            rhs=QT[:, qstart + a - off: qstart + b - off],
                            start=True, stop=True,
                        )
                    offs.append(off)
                    off += qlen
                P = ppool.tile([128, 1024], DT, tag="P",
                               name=f"P{hl}_{qh}_{g[0][0]}")
                nc.scalar.activation(P[:, 0:off], St[:, 0:off], EXP, scale=0.125)
                for (kc, qstart, qlen), o in zip(g, offs):
                    if kc * 128 == qstart:
                        nc.vector.tensor_mul(
                            P[:, o:o + 128], P[:, o:o + 128], mask01)
                    rel = qstart - qh * 1024
                    for (a, b) in _pieces(rel, rel + qlen):
                        nc.tensor.matmul(
                            outp[a // 512][:, a % 512:a % 512 + b - a],
                            lhsT=Vc[:, kc, :],
                            rhs=P[:, o + a - rel: o + b - rel],
                            start=(kc == 0), stop=(kc == bank_last[a // 512]),
                        )

        # ---- softmax-denominator normalization chains.
        # Each (hl, qh) produces a chain: den row PSUM->DRAM, partition-spread
        # reload, DVE reciprocal, DRAM respread, broadcast reload, then the
        # con write (on GpSimd, off the DVE queue). The chain is emitted in
        # three slices across later flush points so no in-order engine queue
        # ever waits on a DMA round-trip hop in flight.
        chains = []

        def flush_chains():
            for ch in list(chains):
                ch.pop(0)()
                if not ch:
                    chains.remove(ch)

        def emit_attention(hl, qh):
            QT, KT, VT, Vc = preps[hl]
            con = conA if hl < 2 else conB
            r0 = 64 * (hl % 2)
            outpt = ops.tile([65, 1024], F32, tag="outp", space="PSUM",
                             name=f"outp{hl}_{qh}")
            outp = [outpt[:, 0:512], outpt[:, 512:1024]]
            if pv_fp8:
                emit_attention_fp8(hl, qh, outp, KT, QT, Vc)
            else:
                emit_attention_bf16(hl, qh, outp, KT, QT, Vc)
            stg = small.tile([65, 1024], F32, tag="stg", name=f"stg{hl}_{qh}")
            nc.vector.tensor_copy(stg, outpt)
            # denominator row, partition-spread via DRAM (a (1,1024)
            # single-lane DVE reciprocal costs 6.5us; spread across 128
            # partitions it is ~60ns).
            d_dram = dpool.tile([1, 1024], F32, tag="d_dram", name=f"dd{hl}_{qh}")
            nc.sync.dma_start(out=d_dram, in_=stg[64:65, :])
            spread = small.tile([128, 8], F32, tag="spread", name=f"sp{hl}_{qh}")
            nc.sync.dma_start(
                out=spread,
                in_=d_dram.rearrange("a (p i) -> p a i", p=128)[:, 0, :],
            )
            box = {}

            def s2():
                rspread = small.tile([128, 8], F32, tag="rspread",
                                     name=f"rs{hl}_{qh}")
                nc.vector.reciprocal(rspread, spread)
                r_dram = dpool.tile([1, 1024], F32, tag="r_dram",
                                    name=f"rd{hl}_{qh}")
                nc.sync.dma_start(
                    out=r_dram.rearrange("a (p i) -> p a i", p=128)[:, 0, :],
                    in_=rspread,
                )
                rec64 = small.tile([64, 1024], F32, tag="rec64",
                                   name=f"r64{hl}_{qh}")
                rec_bcast = bass.AP(
                    tensor=r_dram.tensor, offset=r_dram.offset,
                    ap=[[0, 64]] + [list(d) for d in r_dram.ap[1:]],
                )
                nc.sync.dma_start(out=rec64, in_=rec_bcast)
                box["rec64"] = rec64

            def s3():
                # stg/rec64 columns are chunk-major permuted; iterate in true
                # pseudo-position order (innermost 16 contiguous) so con
                # comes out unpermuted for the output projection.
                perm = "d (qc jb sq) -> d qc sq jb"
                nc.gpsimd.tensor_mul(
                    con[r0:r0 + 64, qh * 1024:(qh + 1) * 1024].rearrange(
                        "d (qc sq jb) -> d qc sq jb", qc=8, sq=8),
                    stg[0:64, :].rearrange(perm, qc=8, jb=16),
                    box["rec64"].rearrange(perm, qc=8, jb=16),
                )

            chains.append([s2, s3])

        for hl in range(HPC):
            emit_prep(hl)
            for qh in range(2):
                emit_attention(hl, qh)
                flush_chains()
            preps.pop(hl)
        while chains:
            flush_chains()
        attn_ctx.close()
        oproj = ctx.enter_context(tc.tile_pool(name="oproj", bufs=5, space="PSUM"))

        # ---- output projection: partialT[j, s] = woutT_s.T @ [conA; conB]
        # PSUM->SBUF staging copies alternate ACT/DVE so neither engine's
        # ~0.5us-per-tile copy serializes the 32-tile drain.
        # sb-outer: the sb=0,1 tiles only need the qh=0 halves of con, which
        # are ready before the last qh=1 normalization chains drain.
        for i, (sb, jc) in enumerate(
                (sb, jc) for sb in range(4) for jc in range(8)):
            ps = oproj.tile([128, 512], F32, tag="op")
            nc.tensor.matmul(
                ps, lhsT=wout_sb[:, 0, jc * 128:(jc + 1) * 128],
                rhs=conA[:, sb * 512:(sb + 1) * 512],
                start=True, stop=False,
            )
            nc.tensor.matmul(
                ps, lhsT=wout_sb[:, 1, jc * 128:(jc + 1) * 128],
                rhs=conB[:, sb * 512:(sb + 1) * 512],
                start=False, stop=True,
            )
            st = stpool.tile([128, 512], BF16, tag="st")
            if i % 2 == 0:
                nc.scalar.copy(st, ps)
            else:
                nc.vector.tensor_copy(st, ps)
            r = (sb * 8 + jc) * 128
            nc.sync.dma_start(out=outT[r:r + 128, :], in_=st)
    nc.compile()
    return nc


def make_in_maps(x, Wqkv, bqkv, Wout, mm_dt=BF16, pv_fp8=False):
    np_dt = mybir.dt.np(mm_dt)
    x = np.asarray(x, np.float32)
    xT = np.ascontiguousarray(x.transpose(0, 2, 1)).astype(np_dt)  # (2,1024,2048)
    WqkvT = np.asarray(Wqkv, np.float32).T.astype(np_dt)
    WoutT = np.asarray(Wout, np.float32).T.astype(np_dt)
    bq = np.asarray(bqkv, np.float32).reshape(1, 3 * E)
    # wq[(t*8+ec)*128+p, c] = WqkvT[ec*128+p, t*1024+c]
    wqh = np.ascontiguousarray(
        WqkvT.reshape(8, 128, 3, 1024).transpose(2, 0, 1, 3).reshape(3072, 1024))
    # mask for the diagonal 128-chunk: key partitions AND query columns are
    # both in within-chunk scatter order (idx = 8*jb + s8 -> true 16*s8 + jb).
    p = np.arange(128)
    pos = 16 * (p % 8) + p // 8
    allowed = pos[:, None] <= pos[None, :]
    if pv_fp8:
        maskp = np.where(allowed, 0.0, -1e9).astype(np.float32)
    else:
        maskp = allowed.astype(np_dt)
    in_maps = []
    for c in range(8):
        b, qd = divmod(c, 4)
        xc = xT[b][:, qd * SL:(qd + 1) * SL]  # [1024, 512]
        # xq[hl*128+p, ec*128+s] = xc[ec*128+p, hl*128+s]
        xqh = np.ascontiguousarray(
            xc.reshape(8, 128, 4, 128).transpose(2, 1, 0, 3).reshape(512, 1024))
        wo = WoutT[qd * 256:(qd + 1) * 256, :]  # [256, 1024]
        woq = np.ascontiguousarray(
            wo.reshape(2, 128, E).transpose(1, 0, 2).reshape(128, 2 * E))
        in_maps.append({
            "xq": xqh,
            "wq": wqh,
            "bqkv": bq,
            "woutq": woq,
            "maskp": maskp,
        })
    return in_maps


_NC_CACHE = {}


def get_program(mm_dt=BF16, pv_fp8=False):
    key = (str(mm_dt), pv_fp8)
    if key not in _NC_CACHE:
        _NC_CACHE[key] = build_program(mm_dt, pv_fp8)
    return _NC_CACHE[key]


def assemble(results, bout):
    bout = np.asarray(bout, np.float32)
    out = np.zeros((B, S, E), np.float32)
    for c in range(8):
        b = c // 4
        # tile-major [sb, jc, 128, 512] -> [E, S] -> transpose to [S, E]
        pt = results[c]["partialT"].reshape(4, 8, 128, 512).astype(np.float32)
        out[b] += pt.transpose(0, 3, 1, 2).reshape(S, E)
    out += bout
    return out


def kernel(x, Wqkv, bqkv, Wout, bout, mm_dt=BF16, pv_fp8=False, trace=False):
    nc = get_program(mm_dt, pv_fp8)
    in_maps = make_in_maps(x, Wqkv, bqkv, Wout, mm_dt, pv_fp8)
    res = run_bass_kernel_spmd(nc, in_maps, list(range(8)), trace=trace)
    out = assemble(res.results, bout)
    if trace:
        kernel.last_result = res
    return out



# revision 76
# speedup vs baseline: 1.0104x; 1.0104x over previous
"""Trainium2 Bass kernel for nn_MultiHeadAttention_67018669687091.

Problem: MHA with B=2, S=2048, E=1024, H=16, D=64, causal, fp32.
The reference reshapes (B,S,E)->(B,H,S,D) WITHOUT transpose, so head h of
batch b is the contiguous 128-row x-block rows [h*128,(h+1)*128) viewed as a
(2048, 64) pseudo-sequence: position 16*s+j <- (row s, channel 64j+d).

Sharding: 8 cores; core c owns batch b=c//4 and head-quad qd=c%4 (4 heads).
Each core computes the qkv projection for its 4 blocks, per-head causal
attention in the transposed domain (scores with key on partitions, softmax
along the free dim via an augmented ones-row of V and late normalization),
and a row-parallel slice of the output projection. Host sums the 4 partials
per batch and adds bout.

Performance notes (per-core, PE-column-bound; the PE clock is power-governed
so matmul column count is the controlling resource):
 - QT/KT/VT are stored chunk-major permuted (col = 128*kc + 8*jb + s8) so
   the qkv->per-head scatter copies are contiguous-innermost on the DVE
   (4.6us -> ~1.2us per copy); all causal chunk slicing is unchanged, the
   host-built mask handles the within-chunk order, and the permutation is
   undone for free in the con write.
 - Inputs are host-prepacked tile-contiguous; x/bias issue from the Scalar
   HWDGE queue in parallel with weights on Sync (dma_start issue time, not
   bandwidth, gates the prologue). The partial output is written tile-major.
 - Softmax denominator reciprocal: partition-spread via a DRAM round trip,
   emitted in deferred slices across later flush points so no in-order
   engine queue waits on a DMA hop; the con multiply runs on GpSimd.
 - Attention-phase PSUM pools close before the output projection so it gets
   a 5-deep pool (with 2 bufs each tile's matmuls serialize behind the
   previous tile's staging copy).
 - fp8 DoubleRow PV (pv_fp8=True) works but is a net loss: the power
   governor tracks MAC-rate, which DoubleRow does not reduce, and exp
   overflows e4m3 without per-head max subtraction (NaN). Kept for
   reference, off by default.

Matmul operand dtype is selectable: bfloat16 (fastest), float32r, float32.
"""
import numpy as np
from contextlib import ExitStack

import concourse.bass as bass
import concourse.bacc as bacc
import concourse.mybir as mybir
import concourse.tile as tile
from concourse.masks import make_identity
from concourse.bass_utils import run_bass_kernel_spmd

E = 1024
H = 16
D = 64
B = 2
S = 2048
HPC = 4          # heads per core
SL = HPC * 128   # x columns per core (512)

F32 = mybir.dt.float32
F32R = mybir.dt.float32r
BF16 = mybir.dt.bfloat16
FP8 = mybir.dt.float8e4
EXP = mybir.ActivationFunctionType.Exp
DR = mybir.MatmulPerfMode.DoubleRow


def _pieces(lo, hi, bank=512):
    """Split [lo, hi) at multiples of `bank` (PSUM bank boundaries)."""
    out = []
    while lo < hi:
        nxt = min(hi, (lo // bank + 1) * bank)
        out.append((lo, nxt))
        lo = nxt
    return out


def build_program(mm_dt=BF16, pv_fp8=False):
    """One SPMD program; per-core data comes via in_maps."""
    nc = bacc.Bacc("TRN2", target_bir_lowering=False)
    DT = mm_dt
    # dtype of the qkv tiles / transposes: must pair legally with identity
    QKVDT = DT if DT == BF16 else F32

    # Inputs host-prepacked so that every load is a single clean 2D DMA with
    # contiguous per-partition lines (cheap Sync-issue descriptors):
    #   xq[hl*128+p, ec*128+s]      = x[b].T[ec*128+p, hl*128+s]
    #   wq[(t*8+ec)*128+p, c]       = Wqkv.T[ec*128+p, t*1024+c]
    #   woutq[p, hf*E+j]            = Wout.T[qd*256+hf*128+p, j]
    xq = nc.dram_tensor("xq", [HPC * 128, 1024], DT, kind="ExternalInput").ap()
    wq = nc.dram_tensor("wq", [24 * 128, 1024], DT, kind="ExternalInput").ap()
    bqkv = nc.dram_tensor("bqkv", [1, 3 * E], F32, kind="ExternalInput").ap()
    woutq = nc.dram_tensor("woutq", [128, 2 * E], DT, kind="ExternalInput").ap()
    maskd = nc.dram_tensor(
        "maskp", [128, 128], F32 if pv_fp8 else BF16, kind="ExternalInput").ap()
    # tile-major output: row block (sb*8+jc) holds the [128, 512] tile for
    # out channels jc*128.. and positions sb*512.. — every DMA write lands
    # fully contiguous in DRAM; the host assembles the final layout.
    outT = nc.dram_tensor("partialT", [32 * 128, 512], BF16,
                          kind="ExternalOutput").ap()

    with tile.TileContext(nc) as tc, ExitStack() as ctx:
        singles = ctx.enter_context(tc.tile_pool(name="singles", bufs=1))
        wpool = ctx.enter_context(tc.tile_pool(name="wpool", bufs=2))
        qkvpool = ctx.enter_context(tc.tile_pool(name="qkvpool", bufs=1))
        headpool = ctx.enter_context(tc.tile_pool(name="headpool", bufs=2))
        ppool = ctx.enter_context(tc.tile_pool(name="ppool", bufs=4))
        cpool = ctx.enter_context(tc.tile_pool(name="cpool", bufs=1))
        stpool = ctx.enter_context(tc.tile_pool(name="stpool", bufs=3))
        small = ctx.enter_context(tc.tile_pool(name="small", bufs=3))
        dpool = ctx.enter_context(tc.tile_pool(name="dpool", bufs=2, space="DRAM"))
        mmps = ctx.enter_context(tc.tile_pool(name="mmps", bufs=2, space="PSUM"))
        # attention-phase PSUM pools live in their own scope: they close
        # before the output projection so their 6 banks can be reused for a
        # deeply-buffered out-proj pool (bufs=2 there serializes each tile's
        # matmuls behind the previous tile's staging copy).
        attn_ctx = ExitStack()
        sps = attn_ctx.enter_context(tc.tile_pool(name="sps", bufs=2, space="PSUM"))
        ops = attn_ctx.enter_context(tc.tile_pool(name="ops", bufs=1, space="PSUM"))

        ident = singles.tile([128, 128], QKVDT, tag="ident")
        make_identity(nc, ident)

        # Diagonal-chunk causal mask, host-computed for the permuted key/query
        # order (idx = 8*jb + s8 -> pos 16*s8 + jb). pv_fp8: additive 0/-1e9
        # f32 mask applied to scores pre-exp. bf16: multiplicative 0/1 on P.
        # (loaded after the projection emission — not startup-critical)
        mask01 = singles.tile([128, 128], F32 if pv_fp8 else BF16, tag="mask01")

        # Startup-critical loads only: x tiles and the qkv bias. wout and the
        # mask are deferred until after the projection loop — every dma_start
        # costs ~0.65us of serial issue time on the Sync queue, so front
        # issue-count is what sets when the first matmul can run.
        # x + bias issue from the (startup-idle) Scalar queue, weights from
        # Sync — the two HWDGE issuers run in parallel, halving the serial
        # issue time in front of the first projection matmul.
        xts = []
        for hl in range(HPC):
            xt = singles.tile([128, 8, 128], DT, tag=f"xt{hl}")
            nc.scalar.dma_start(out=xt, in_=xq[hl * 128:(hl + 1) * 128, :])
            xts.append(xt)

        bias_sb = singles.tile([128, 3 * E], F32, tag="bias")
        bq_bcast = bass.AP(
            tensor=bqkv.tensor, offset=bqkv.offset,
            ap=[[0, 128]] + [list(d) for d in bqkv.ap[1:]],
        )
        nc.scalar.dma_start(out=bias_sb, in_=bq_bcast)

        # ---- QKV projection: qkv[hl] = x_blk @ WqkvT + bqkv  (128, 3072)
        qkvs = [
            qkvpool.tile([128, 3 * E], QKVDT, tag=f"qkv{hl}", name=f"qkv{hl}")
            for hl in range(HPC)
        ]

        # ---- per-head attention prep machinery. qkv columns of tensor t
        # (q/k/v) are exactly proj nb-blocks (2t, 2t+1), so each tensor's
        # transposes + scatter copies are emitted right after its two proj
        # blocks: the DVE scatter stream for heads 0/1 runs during the
        # projection (where DVE is otherwise idle) instead of after it.
        conA = cpool.tile([128, S], DT, tag="conA")
        conB = cpool.tile([128, S], DT, tag="conB")
        preps = {}

        def head_tiles(hl):
            if hl not in preps:
                # QT/KT zero-padded to 128 partitions: K=128 full-array S
                # matmuls stream at 216ns (K=64 serializes LDWEIGHTS).
                QT = headpool.tile([128, S], DT, tag="QT", name=f"QT{hl}")
                KT = headpool.tile([128, S], DT, tag="KT", name=f"KT{hl}")
                VT = headpool.tile([65, S], QKVDT, tag="VT", name=f"VT{hl}")
                # pv_fp8: inner dim padded to 80 — DoubleRow LDWEIGHTS requires
                # the K-pair stride to be a multiple of 16.
                Vc = headpool.tile([128, 16, 80 if pv_fp8 else 65],
                                   FP8 if pv_fp8 else DT,
                                   tag="Vc", name=f"Vc{hl}")
                if hl < 2:
                    # pads/ones are written once per pool slot (bufs=2, slots
                    # alternate hl%2); heads 2/3 reuse them — scatters only
                    # touch rows 0:64, so the constant rows persist and the
                    # per-head memset (plus its WAR serialization in the prep
                    # chain) is skipped.
                    nc.gpsimd.memset(QT[64:128, :], 0.0)
                    nc.gpsimd.memset(KT[64:128, :], 0.0)
                    nc.gpsimd.memset(VT[64:65, :], 1.0)
                preps[hl] = [QT, KT, VT, Vc]
            return preps[hl]

        def emit_tensor_prep(hl, t):
            # QT/KT/VT hold the head's (d, pseudo-seq) slab chunk-major:
            # column 128*kc + 8*jb + s8, for pseudo-position 16*(8*kc+s8)+jb.
            # The permutation is local to each 128-chunk, so all causal
            # chunk/piece slicing is unchanged; the scatter copy below gets a
            # contiguous innermost run (fast on DVE) instead of a stride-16
            # element scatter (4.6us -> ~1.2us per copy). Scores/P/outp/stg
            # inherit the within-chunk column order; it is undone in the con
            # write, and the host-built mask01 accounts for it.
            dest = head_tiles(hl)[t]
            tpb = mmps.tile([128, 1024], QKVDT, tag="mm", space="PSUM",
                            name=f"tpb{hl}_{t}")
            for cc in range(8):
                nc.tensor.transpose(
                    tpb[:, cc * 128:(cc + 1) * 128],
                    qkvs[hl][:, t * 1024 + cc * 128: t * 1024 + (cc + 1) * 128],
                    ident,
                )
            t3 = tpb.rearrange("p (cc s) -> p cc s", cc=8)
            for jp in range(2):
                csrc = t3[64 * jp:64 * jp + 64, :, :].rearrange(
                    "d cc (kc sk) -> d cc kc sk", kc=16)
                dd = dest[0:64].rearrange(
                    "d (kc cc two sk) -> d two cc kc sk",
                    kc=16, cc=8, two=2)[:, jp, :, :, :]
                nc.vector.tensor_copy(dd, csrc)

        def emit_vc_prep(hl):
            QT, KT, VT, Vc = head_tiles(hl)
            for kc in range(16):
                tp = mmps.tile([128, 512], QKVDT, tag="mm", space="PSUM",
                               name=f"tpv{hl}_{kc}")
                nc.tensor.transpose(
                    tp[:, 0:65], VT[:, kc * 128:(kc + 1) * 128], ident[0:65, 0:65])
                nc.vector.tensor_copy(Vc[:, kc, 0:65], tp[:, 0:65])

        def emit_prep(hl):
            for t in range(3):
                emit_tensor_prep(hl, t)
            emit_vc_prep(hl)

        # ---- QKV projection. Weights come in as one [128, 1024] DMA per
        # (t, ec) covering both nb blocks of tensor t: halves the number of
        # Sync-issued dma_starts on the startup-critical path.
        for t in range(3):
            wts = []
            for ec in range(8):
                wtc = wpool.tile([128, 1024], DT, tag=f"wt{ec}", name=f"wt{t}_{ec}")
                r = (t * 8 + ec) * 128
                nc.sync.dma_start(out=wtc, in_=wq[r:r + 128, :])
                wts.append(wtc)
            for nb in (2 * t, 2 * t + 1):
                h = (nb % 2) * 512
                for hl in range(HPC):
                    ps = mmps.tile([128, 512], F32, tag="mm")
                    for ec in range(8):
                        nc.tensor.matmul(
                            ps, lhsT=xts[hl][:, ec, :], rhs=wts[ec][:, h:h + 512],
                            start=(ec == 0), stop=(ec == 7),
                        )
                    nc.vector.tensor_add(
                        qkvs[hl][:, nb * 512:(nb + 1) * 512], ps,
                        bias_sb[:, nb * 512:(nb + 1) * 512],
                    )

        # deferred non-critical input loads (used from attention onward)
        nc.sync.dma_start(out=mask01, in_=maskd)
        wout_sb = singles.tile([128, 2, E], DT, tag="wout")
        nc.sync.dma_start(out=wout_sb, in_=woutq.rearrange("p (hf j) -> p hf j", hf=2))

        def emit_scores(St, kc, qstart, qlen, KT, QT):
            for (a, b) in _pieces(0, qlen):
                nc.tensor.matmul(
                    St[:, a:b],
                    lhsT=KT[:, kc * 128:(kc + 1) * 128],
                    rhs=QT[:, qstart + a: qstart + b],
                    start=True, stop=True,
                )
            if kc * 128 == qstart and pv_fp8:
                # additive -1e9 mask on the diagonal chunk, pre-exp
                nc.vector.tensor_add(St[:, 0:128], St[:, 0:128], mask01)

        def emit_attention_fp8(hl, qh, outp, KT, QT, Vc):
            # Key chunks processed in pairs (2c, 2c+1); P stored as fp8 planes
            # and PV runs as K=256 DoubleRow matmuls (half the PE columns).
            # The even chunk's extra 128 queries (diagonal sliver) get a
            # separate plain-fp8 matmul.
            npairs = 4 * (qh + 1)
            relPs = [max(128 * (2 * c + 1) - 1024 * qh, 0) for c in range(npairs)]
            bank_last = {
                bk: max(c for c in range(npairs) if relPs[c] < 512 * (bk + 1))
                for bk in range(2)
            }
            for c in range(npairs):
                ke, ko = 2 * c, 2 * c + 1
                qs_e = max(ke * 128, qh * 1024)
                qs_o = max(ko * 128, qh * 1024)
                sliver = qs_o - qs_e  # 0 or 128
                qlen_e = (qh + 1) * 1024 - qs_e
                qlen_o = (qh + 1) * 1024 - qs_o
                P8t = ppool.tile([128, 2, 1024], FP8, tag="P",
                                 name=f"P{hl}_{qh}_{c}")
                for (kk, qs, qlen, pl) in ((ke, qs_e, qlen_e, 0),
                                           (ko, qs_o, qlen_o, 1)):
                    St = sps.tile([128, 1024], F32, tag="S", space="PSUM",
                                  name=f"St{hl}_{qh}_{kk}")
                    emit_scores(St, kk, qs, qlen, KT, QT)
                    nc.scalar.activation(P8t[:, pl, 0:qlen], St[:, 0:qlen],
                                         EXP, scale=0.125)
                relP = relPs[c]
                if sliver:
                    a = relP - 128
                    nc.tensor.matmul(
                        outp[a // 512][:, a % 512:a % 512 + 128],
                        lhsT=Vc[:, ke, 0:65],
                        rhs=P8t[:, 0, 0:128],
                        start=(c == 0), stop=False,
                    )
                for (a, b) in _pieces(relP, relP + qlen_o):
                    rhs = bass.AP(
                        tensor=P8t.tensor,
                        offset=P8t.offset + sliver + (a - relP),
                        ap=[list(P8t.ap[0]), [1024 - sliver, 2], [1, b - a]],
                    )
                    nc.tensor.matmul(
                        outp[a // 512][:, a % 512:a % 512 + b - a],
                        lhsT=Vc[:, ke:ke + 2, 0:65],
                        rhs=rhs,
                        start=(c == 0), stop=(c == bank_last[a // 512]),
                        perf_mode=DR,
                    )

        def emit_attention_bf16(hl, qh, outp, KT, QT, Vc):
            # Key chunks whose query lengths sum to 1024 share one St tile
            # and ONE exp: the causal staircase pairs up exactly (896+128,
            # 768+256, 640+384, ...), cutting ACTIVATE count from 24 to 18
            # per head (~300ns fixed overhead each, and fewer exp-latency
            # ping-pong points for the PE).
            items = []
            for kc in range(8 * (qh + 1)):
                qstart = max(kc * 128, qh * 1024)
                items.append((kc, qstart, (qh + 1) * 1024 - qstart))
            full = [[it] for it in items if it[2] >= 1024]
            rest = sorted((it for it in items if it[2] < 1024),
                          key=lambda it: -it[2])
            groups = list(full)
            i, j = 0, len(rest) - 1
            while i <= j:
                if i < j and rest[i][2] + rest[j][2] <= 1024:
                    groups.append([rest[i], rest[j]])
                    i, j = i + 1, j - 1
                else:
                    groups.append([rest[i]])
                    i += 1
            groups.sort(key=lambda g: min(it[0] for it in g))
            # per-PSUM-bank last writer under the actual emission order
            bank_last = {}
            for g in groups:
                for (kc, qstart, qlen) in g:
                    rel = qstart - qh * 1024
                    for (a, b) in _pieces(rel, rel + qlen):
                        bank_last[a // 512] = kc
            for g in groups:
                St = sps.tile([128, 1024], F32, tag="S", space="PSUM",
                              name=f"St{hl}_{qh}_{g[0][0]}")
                off, offs = 0, []
                for (kc, qstart, qlen) in g:
                    for (a, b) in _pieces(off, off + qlen):
                        nc.tensor.matmul(
                            St[:, a:b],
                            lhsT=KT[:, kc * 128:(kc + 1) * 128],
                            rhs=QT[:, qstart + a - off: qstart + b - off],
                            start=True, stop=True,
                        )
                    offs.append(off)
                    off += qlen
                P = ppool.tile([128, 1024], DT, tag="P",
                               name=f"P{hl}_{qh}_{g[0][0]}")
                nc.scalar.activation(P[:, 0:off], St[:, 0:off], EXP, scale=0.125)
                for (kc, qstart, qlen), o in zip(g, offs):
                    if kc * 128 == qstart:
                        nc.vector.tensor_mul(
                            P[:, o:o + 128], P[:, o:o + 128], mask01)
                    rel = qstart - qh * 1024
                    for (a, b) in _pieces(rel, rel + qlen):
                        nc.tensor.matmul(
                            outp[a // 512][:, a % 512:a % 512 + b - a],
                            lhsT=Vc[:, kc, :],
                            rhs=P[:, o + a - rel: o + b - rel],
                            start=(kc == 0), stop=(kc == bank_last[a // 512]),
                        )

        # ---- softmax-denominator normalization chains.
        # Each (hl, qh) produces a chain: den row PSUM->DRAM, partition-spread
        # reload, DVE reciprocal, DRAM respread, broadcast reload, then the
        # con write (on GpSimd, off the DVE queue). The chain is emitted in
        # three slices across later flush points so no in-order engine queue
        # ever waits on a DMA round-trip hop in flight.
        chains = []

        def flush_chains():
            for ch in list(chains):
                ch.pop(0)()
                if not ch:
                    chains.remove(ch)

        def emit_attention(hl, qh):
            QT, KT, VT, Vc = preps[hl]
            con = conA if hl < 2 else conB
            r0 = 64 * (hl % 2)
            outpt = ops.tile([65, 1024], F32, tag="outp", space="PSUM",
                             name=f"outp{hl}_{qh}")
            outp = [outpt[:, 0:512], outpt[:, 512:1024]]
            if pv_fp8:
                emit_attention_fp8(hl, qh, outp, KT, QT, Vc)
            else:
                emit_attention_bf16(hl, qh, outp, KT, QT, Vc)
            stg = small.tile([65, 1024], F32, tag="stg", name=f"stg{hl}_{qh}")
            nc.vector.tensor_copy(stg, outpt)
            # denominator row, partition-spread via DRAM (a (1,1024)
            # single-lane DVE reciprocal costs 6.5us; spread across 128
            # partitions it is ~60ns).
            d_dram = dpool.tile([1, 1024], F32, tag="d_dram", name=f"dd{hl}_{qh}")
            nc.sync.dma_start(out=d_dram, in_=stg[64:65, :])
            spread = small.tile([128, 8], F32, tag="spread", name=f"sp{hl}_{qh}")
            nc.sync.dma_start(
                out=spread,
                in_=d_dram.rearrange("a (p i) -> p a i", p=128)[:, 0, :],
            )
            box = {}

            def s2():
                rspread = small.tile([128, 8], F32, tag="rspread",
                                     name=f"rs{hl}_{qh}")
                nc.vector.reciprocal(rspread, spread)
                r_dram = dpool.tile([1, 1024], F32, tag="r_dram",
                                    name=f"rd{hl}_{qh}")
                nc.sync.dma_start(
                    out=r_dram.rearrange("a (p i) -> p a i", p=128)[:, 0, :],
                    in_=rspread,
                )
                rec64 = small.tile([64, 1024], F32, tag="rec64",
                                   name=f"r64{hl}_{qh}")
                rec_bcast = bass.AP(
                    tensor=r_dram.tensor, offset=r_dram.offset,
                    ap=[[0, 64]] + [list(d) for d in r_dram.ap[1:]],
                )
                nc.sync.dma_start(out=rec64, in_=rec_bcast)
                box["rec64"] = rec64

            def s3():
                # stg/rec64 columns are chunk-major permuted; iterate in true
                # pseudo-position order (innermost 16 contiguous) so con
                # comes out unpermuted for the output projection.
                perm = "d (qc jb sq) -> d qc sq jb"
                nc.gpsimd.tensor_mul(
                    con[r0:r0 + 64, qh * 1024:(qh + 1) * 1024].rearrange(
                        "d (qc sq jb) -> d qc sq jb", qc=8, sq=8),
                    stg[0:64, :].rearrange(perm, qc=8, jb=16),
                    box["rec64"].rearrange(perm, qc=8, jb=16),
                )

            chains.append([s2, s3])

        for hl in range(HPC):
            emit_prep(hl)
            for qh in range(2):
                emit_attention(hl, qh)
                flush_chains()
            preps.pop(hl)
        while chains:
            flush_chains()
        attn_ctx.close()
        oproj = ctx.enter_context(tc.tile_pool(name="oproj", bufs=5, space="PSUM"))

        # ---- output projection: partialT[j, s] = woutT_s.T @ [conA; conB]
        # PSUM->SBUF staging copies alternate ACT/DVE so neither engine's
        # ~0.5us-per-tile copy serializes the 32-tile drain.
        # sb-outer: the sb=0,1 tiles only need the qh=0 halves of con, which
        # are ready before the last qh=1 normalization chains drain.
        for i, (sb, jc) in enumerate(
                (sb, jc) for sb in range(4) for jc in range(8)):
            ps = oproj.tile([128, 512], F32, tag="op")
            nc.tensor.matmul(
                ps, lhsT=wout_sb[:, 0, jc * 128:(jc + 1) * 128],
                rhs=conA[:, sb * 512:(sb + 1) * 512],
                start=True, stop=False,
            )
            nc.tensor.matmul(
                ps, lhsT=wout_sb[:, 1, jc * 128:(jc + 1) * 128],
                rhs=conB[:, sb * 512:(sb + 1) * 512],
                start=False, stop=True,
            )
            st = stpool.tile([128, 512], BF16, tag="st")
            if i % 2 == 0:
                nc.scalar.copy(st, ps)
            else:
                nc.vector.tensor_copy(st, ps)
            r = (sb * 8 + jc) * 128
            nc.sync.dma_start(out=outT[r:r + 128, :], in_=st)
    nc.compile()
    return nc


def make_in_maps(x, Wqkv, bqkv, Wout, mm_dt=BF16, pv_fp8=False):
    np_dt = mybir.dt.np(mm_dt)
    x = np.asarray(x, np.float32)
    xT = np.ascontiguousarray(x.transpose(0, 2, 1)).astype(np_dt)  # (2,1024,2048)
    WqkvT = np.asarray(Wqkv, np.float32).T.astype(np_dt)
    WoutT = np.asarray(Wout, np.float32).T.astype(np_dt)
    bq = np.asarray(bqkv, np.float32).reshape(1, 3 * E)
    # wq[(t*8+ec)*128+p, c] = WqkvT[ec*128+p, t*1024+c]
    wqh = np.ascontiguousarray(
        WqkvT.reshape(8, 128, 3, 1024).transpose(2, 0, 1, 3).reshape(3072, 1024))
    # mask for the diagonal 128-chunk: key partitions AND query columns are
    # both in within-chunk scatter order (idx = 8*jb + s8 -> true 16*s8 + jb).
    p = np.arange(128)
    pos = 16 * (p % 8) + p // 8
    allowed = pos[:, None] <= pos[None, :]
    if pv_fp8:
        maskp = np.where(allowed, 0.0, -1e9).astype(np.float32)
    else:
        maskp = allowed.astype(np_dt)
    in_maps = []
    for c in range(8):
        b, qd = divmod(c, 4)
        xc = xT[b][:, qd * SL:(qd + 1) * SL]  # [1024, 512]
        # xq[hl*128+p, ec*128+s] = xc[ec*128+p, hl*128+s]
        xqh = np.ascontiguousarray(
            xc.reshape(8, 128, 4, 128).transpose(2, 1, 0, 3).reshape(512, 1024))
        wo = WoutT[qd * 256:(qd + 1) * 256, :]  # [256, 1024]
        woq = np.ascontiguousarray(
            wo.reshape(2, 128, E).transpose(1, 0, 2).reshape(128, 2 * E))
        in_maps.append({
            "xq": xqh,
            "wq": wqh,
            "bqkv": bq,
            "woutq": woq,
            "maskp": maskp,
        })
    return in_maps


_NC_CACHE = {}


def get_program(mm_dt=BF16, pv_fp8=False):
    key = (str(mm_dt), pv_fp8)
    if key not in _NC_CACHE:
        _NC_CACHE[key] = build_program(mm_dt, pv_fp8)
    return _NC_CACHE[key]


def assemble(results, bout):
    bout = np.asarray(bout, np.float32)
    out = np.zeros((B, S, E), np.float32)
    for c in range(8):
        b = c // 4
        # tile-major [sb, jc, 128, 512] -> [E, S] -> transpose to [S, E]
        pt = results[c]["partialT"].reshape(4, 8, 128, 512).astype(np.float32)
        out[b] += pt.transpose(0, 3, 1, 2).reshape(S, E)
    out += bout
    return out


def kernel(x, Wqkv, bqkv, Wout, bout, mm_dt=BF16, pv_fp8=False, trace=False):
    nc = get_program(mm_dt, pv_fp8)
    in_maps = make_in_maps(x, Wqkv, bqkv, Wout, mm_dt, pv_fp8)
    res = run_bass_kernel_spmd(nc, in_maps, list(range(8)), trace=trace)
    out = assemble(res.results, bout)
    if trace:
        kernel.last_result = res
    return out



# revision 77
# speedup vs baseline: 1.0515x; 1.0407x over previous
"""Trainium2 Bass kernel for nn_MultiHeadAttention_67018669687091.

Problem: MHA with B=2, S=2048, E=1024, H=16, D=64, causal, fp32.
The reference reshapes (B,S,E)->(B,H,S,D) WITHOUT transpose, so head h of
batch b is the contiguous 128-row x-block rows [h*128,(h+1)*128) viewed as a
(2048, 64) pseudo-sequence: position 16*s+j <- (row s, channel 64j+d).

Sharding: 8 cores; core c owns batch b=c//4 and head-quad qd=c%4 (4 heads).
Each core computes the qkv projection for its 4 blocks, per-head causal
attention in the transposed domain (scores with key on partitions, softmax
along the free dim via an augmented ones-row of V and late normalization),
and a row-parallel slice of the output projection. Host sums the 4 partials
per batch and adds bout.

Performance notes (per-core, PE-column-bound; the PE clock is power-governed
so matmul column count is the controlling resource):
 - QT/KT/VT are stored chunk-major permuted (col = 128*kc + 8*jb + s8) so
   the qkv->per-head scatter copies are contiguous-innermost on the DVE
   (4.6us -> ~1.2us per copy); all causal chunk slicing is unchanged, the
   host-built mask handles the within-chunk order, and the permutation is
   undone for free in the con write.
 - Inputs are host-prepacked tile-contiguous; x/bias issue from the Scalar
   HWDGE queue in parallel with weights on Sync (dma_start issue time, not
   bandwidth, gates the prologue). The partial output is written tile-major.
 - Softmax denominator reciprocal: partition-spread via a DRAM round trip,
   emitted in deferred slices across later flush points so no in-order
   engine queue waits on a DMA hop; the con multiply runs on GpSimd.
 - Attention-phase PSUM pools close before the output projection so it gets
   a 5-deep pool (with 2 bufs each tile's matmuls serialize behind the
   previous tile's staging copy).
 - fp8 DoubleRow PV (pv_fp8=True) works but is a net loss: the power
   governor tracks MAC-rate, which DoubleRow does not reduce, and exp
   overflows e4m3 without per-head max subtraction (NaN). Kept for
   reference, off by default.

Matmul operand dtype is selectable: bfloat16 (fastest), float32r, float32.
"""
import numpy as np
from contextlib import ExitStack

import concourse.bass as bass
import concourse.bacc as bacc
import concourse.mybir as mybir
import concourse.tile as tile
from concourse.masks import make_identity
from concourse.bass_utils import run_bass_kernel_spmd

E = 1024
H = 16
D = 64
B = 2
S = 2048
HPC = 4          # heads per core
SL = HPC * 128   # x columns per core (512)

F32 = mybir.dt.float32
F32R = mybir.dt.float32r
BF16 = mybir.dt.bfloat16
FP8 = mybir.dt.float8e4
EXP = mybir.ActivationFunctionType.Exp
DR = mybir.MatmulPerfMode.DoubleRow


def _pieces(lo, hi, bank=512):
    """Split [lo, hi) at multiples of `bank` (PSUM bank boundaries)."""
    out = []
    while lo < hi:
        nxt = min(hi, (lo // bank + 1) * bank)
        out.append((lo, nxt))
        lo = nxt
    return out


def build_program(mm_dt=BF16, pv_fp8=False):
    """One SPMD program; per-core data comes via in_maps."""
    nc = bacc.Bacc("TRN2", target_bir_lowering=False)
    DT = mm_dt
    # dtype of the qkv tiles / transposes: must pair legally with identity
    QKVDT = DT if DT == BF16 else F32

    # Inputs host-prepacked so that every load is a single clean 2D DMA with
    # contiguous per-partition lines (cheap Sync-issue descriptors):
    #   xq[hl*128+p, ec*128+s]      = x[b].T[ec*128+p, hl*128+s]
    #   wq[(t*8+ec)*128+p, c]       = Wqkv.T[ec*128+p, t*1024+c]
    #   woutq[p, hf*E+j]            = Wout.T[qd*256+hf*128+p, j]
    xq = nc.dram_tensor("xq", [HPC * 128, 1024], DT, kind="ExternalInput").ap()
    wq = nc.dram_tensor("wq", [24 * 128, 1024], DT, kind="ExternalInput").ap()
    bqkv = nc.dram_tensor("bqkv", [1, 3 * E], F32, kind="ExternalInput").ap()
    woutq = nc.dram_tensor("woutq", [128, 2 * E], DT, kind="ExternalInput").ap()
    maskd = nc.dram_tensor(
        "maskp", [128, 128], F32 if pv_fp8 else BF16, kind="ExternalInput").ap()
    # tile-major output: row block (sb*8+jc) holds the [128, 512] tile for
    # out channels jc*128.. and positions sb*512.. — every DMA write lands
    # fully contiguous in DRAM; the host assembles the final layout.
    outT = nc.dram_tensor("partialT", [32 * 128, 512], BF16,
                          kind="ExternalOutput").ap()

    with tile.TileContext(nc) as tc, ExitStack() as ctx:
        singles = ctx.enter_context(tc.tile_pool(name="singles", bufs=1))
        wpool = ctx.enter_context(tc.tile_pool(name="wpool", bufs=2))
        qkvpool = ctx.enter_context(tc.tile_pool(name="qkvpool", bufs=1))
        headpool = ctx.enter_context(tc.tile_pool(name="headpool", bufs=2))
        ppool = ctx.enter_context(tc.tile_pool(name="ppool", bufs=4))
        cpool = ctx.enter_context(tc.tile_pool(name="cpool", bufs=1))
        stpool = ctx.enter_context(tc.tile_pool(name="stpool", bufs=3))
        small = ctx.enter_context(tc.tile_pool(name="small", bufs=3))
        dpool = ctx.enter_context(tc.tile_pool(name="dpool", bufs=2, space="DRAM"))
        mmps = ctx.enter_context(tc.tile_pool(name="mmps", bufs=2, space="PSUM"))
        # attention-phase PSUM pools live in their own scope: they close
        # before the output projection so their 6 banks can be reused for a
        # deeply-buffered out-proj pool (bufs=2 there serializes each tile's
        # matmuls behind the previous tile's staging copy).
        attn_ctx = ExitStack()
        sps = attn_ctx.enter_context(tc.tile_pool(name="sps", bufs=2, space="PSUM"))
        ops = attn_ctx.enter_context(tc.tile_pool(name="ops", bufs=1, space="PSUM"))

        ident = singles.tile([128, 128], QKVDT, tag="ident")
        make_identity(nc, ident)

        # Diagonal-chunk causal mask, host-computed for the permuted key/query
        # order (idx = 8*jb + s8 -> pos 16*s8 + jb). pv_fp8: additive 0/-1e9
        # f32 mask applied to scores pre-exp. bf16: multiplicative 0/1 on P.
        # (loaded after the projection emission — not startup-critical)
        mask01 = singles.tile([128, 128], F32 if pv_fp8 else BF16, tag="mask01")

        # Startup-critical loads only: x tiles and the qkv bias. wout and the
        # mask are deferred until after the projection loop — every dma_start
        # costs ~0.65us of serial issue time on the Sync queue, so front
        # issue-count is what sets when the first matmul can run.
        # x + bias issue from the (startup-idle) Scalar queue, weights from
        # Sync — the two HWDGE issuers run in parallel, halving the serial
        # issue time in front of the first projection matmul.
        xts = []
        for hl in range(HPC):
            xt = singles.tile([128, 8, 128], DT, tag=f"xt{hl}")
            nc.scalar.dma_start(out=xt, in_=xq[hl * 128:(hl + 1) * 128, :])
            xts.append(xt)

        bias_sb = singles.tile([128, 3 * E], F32, tag="bias")
        bq_bcast = bass.AP(
            tensor=bqkv.tensor, offset=bqkv.offset,
            ap=[[0, 128]] + [list(d) for d in bqkv.ap[1:]],
        )
        nc.scalar.dma_start(out=bias_sb, in_=bq_bcast)

        # ---- QKV projection: qkv[hl] = x_blk @ WqkvT + bqkv  (128, 3072)
        qkvs = [
            qkvpool.tile([128, 3 * E], QKVDT, tag=f"qkv{hl}", name=f"qkv{hl}")
            for hl in range(HPC)
        ]

        # ---- per-head attention prep machinery. qkv columns of tensor t
        # (q/k/v) are exactly proj nb-blocks (2t, 2t+1), so each tensor's
        # transposes + scatter copies are emitted right after its two proj
        # blocks: the DVE scatter stream for heads 0/1 runs during the
        # projection (where DVE is otherwise idle) instead of after it.
        conA = cpool.tile([128, S], DT, tag="conA")
        conB = cpool.tile([128, S], DT, tag="conB")
        preps = {}

        def head_tiles(hl):
            if hl not in preps:
                # QT/KT zero-padded to 128 partitions: K=128 full-array S
                # matmuls stream at 216ns (K=64 serializes LDWEIGHTS).
                QT = headpool.tile([128, S], DT, tag="QT", name=f"QT{hl}")
                KT = headpool.tile([128, S], DT, tag="KT", name=f"KT{hl}")
                VT = headpool.tile([65, S], QKVDT, tag="VT", name=f"VT{hl}")
                # pv_fp8: inner dim padded to 80 — DoubleRow LDWEIGHTS requires
                # the K-pair stride to be a multiple of 16.
                Vc = headpool.tile([128, 16, 80 if pv_fp8 else 65],
                                   FP8 if pv_fp8 else DT,
                                   tag="Vc", name=f"Vc{hl}")
                if hl < 2:
                    # pads/ones are written once per pool slot (bufs=2, slots
                    # alternate hl%2); heads 2/3 reuse them — scatters only
                    # touch rows 0:64, so the constant rows persist and the
                    # per-head memset (plus its WAR serialization in the prep
                    # chain) is skipped.
                    nc.gpsimd.memset(QT[64:128, :], 0.0)
                    nc.gpsimd.memset(KT[64:128, :], 0.0)
                    nc.gpsimd.memset(VT[64:65, :], 1.0)
                preps[hl] = [QT, KT, VT, Vc]
            return preps[hl]

        def emit_tensor_prep(hl, t):
            # QT/KT/VT hold the head's (d, pseudo-seq) slab chunk-major:
            # column 128*kc + 8*jb + s8, for pseudo-position 16*(8*kc+s8)+jb.
            # The permutation is local to each 128-chunk, so all causal
            # chunk/piece slicing is unchanged; the scatter copy below gets a
            # contiguous innermost run (fast on DVE) instead of a stride-16
            # element scatter (4.6us -> ~1.2us per copy). Scores/P/outp/stg
            # inherit the within-chunk column order; it is undone in the con
            # write, and the host-built mask01 accounts for it.
            dest = head_tiles(hl)[t]
            tpb = mmps.tile([128, 1024], QKVDT, tag="mm", space="PSUM",
                            name=f"tpb{hl}_{t}")
            for cc in range(8):
                nc.tensor.transpose(
                    tpb[:, cc * 128:(cc + 1) * 128],
                    qkvs[hl][:, t * 1024 + cc * 128: t * 1024 + (cc + 1) * 128],
                    ident,
                )
            t3 = tpb.rearrange("p (cc s) -> p cc s", cc=8)
            for jp in range(2):
                csrc = t3[64 * jp:64 * jp + 64, :, :].rearrange(
                    "d cc (kc sk) -> d cc kc sk", kc=16)
                dd = dest[0:64].rearrange(
                    "d (kc cc two sk) -> d two cc kc sk",
                    kc=16, cc=8, two=2)[:, jp, :, :, :]
                nc.vector.tensor_copy(dd, csrc)

        def emit_vc_prep(hl):
            # four chunk-transposes per PSUM tile, drained by ONE strided DVE
            # copy — 4 instead of 16 copies per head, and the PE/DVE
            # ping-pong through the 2 pool slots happens per group of four.
            QT, KT, VT, Vc = head_tiles(hl)
            for g in range(4):
                tp = mmps.tile([128, 1024], QKVDT, tag="mm", space="PSUM",
                               name=f"tpv{hl}_{g}")
                for i in range(4):
                    kc = 4 * g + i
                    nc.tensor.transpose(
                        tp[:, 256 * i:256 * i + 65],
                        VT[:, kc * 128:(kc + 1) * 128], ident[0:65, 0:65])
                nc.vector.tensor_copy(
                    Vc[:, 4 * g:4 * g + 4, 0:65],
                    tp.rearrange("p (i c) -> p i c", i=4)[:, :, 0:65])

        def emit_prep(hl):
            for t in range(3):
                emit_tensor_prep(hl, t)
            emit_vc_prep(hl)

        # ---- QKV projection. Weights come in as one [128, 1024] DMA per
        # (t, ec) covering both nb blocks of tensor t: halves the number of
        # Sync-issued dma_starts on the startup-critical path.
        for t in range(3):
            wts = []
            for ec in range(8):
                wtc = wpool.tile([128, 1024], DT, tag=f"wt{ec}", name=f"wt{t}_{ec}")
                r = (t * 8 + ec) * 128
                nc.sync.dma_start(out=wtc, in_=wq[r:r + 128, :])
                wts.append(wtc)
            for nb in (2 * t, 2 * t + 1):
                h = (nb % 2) * 512
                for hl in range(HPC):
                    ps = mmps.tile([128, 512], F32, tag="mm")
                    for ec in range(8):
                        nc.tensor.matmul(
                            ps, lhsT=xts[hl][:, ec, :], rhs=wts[ec][:, h:h + 512],
                            start=(ec == 0), stop=(ec == 7),
                        )
                    nc.vector.tensor_add(
                        qkvs[hl][:, nb * 512:(nb + 1) * 512], ps,
                        bias_sb[:, nb * 512:(nb + 1) * 512],
                    )

        # deferred non-critical input loads (used from attention onward)
        nc.sync.dma_start(out=mask01, in_=maskd)
        wout_sb = singles.tile([128, 2, E], DT, tag="wout")
        nc.sync.dma_start(out=wout_sb, in_=woutq.rearrange("p (hf j) -> p hf j", hf=2))

        def emit_scores(St, kc, qstart, qlen, KT, QT):
            for (a, b) in _pieces(0, qlen):
                nc.tensor.matmul(
                    St[:, a:b],
                    lhsT=KT[:, kc * 128:(kc + 1) * 128],
                    rhs=QT[:, qstart + a: qstart + b],
                    start=True, stop=True,
                )
            if kc * 128 == qstart and pv_fp8:
                # additive -1e9 mask on the diagonal chunk, pre-exp
                nc.vector.tensor_add(St[:, 0:128], St[:, 0:128], mask01)

        def emit_attention_fp8(hl, qh, outp, KT, QT, Vc):
            # Key chunks processed in pairs (2c, 2c+1); P stored as fp8 planes
            # and PV runs as K=256 DoubleRow matmuls (half the PE columns).
            # The even chunk's extra 128 queries (diagonal sliver) get a
            # separate plain-fp8 matmul.
            npairs = 4 * (qh + 1)
            relPs = [max(128 * (2 * c + 1) - 1024 * qh, 0) for c in range(npairs)]
            bank_last = {
                bk: max(c for c in range(npairs) if relPs[c] < 512 * (bk + 1))
                for bk in range(2)
            }
            for c in range(npairs):
                ke, ko = 2 * c, 2 * c + 1
                qs_e = max(ke * 128, qh * 1024)
                qs_o = max(ko * 128, qh * 1024)
                sliver = qs_o - qs_e  # 0 or 128
                qlen_e = (qh + 1) * 1024 - qs_e
                qlen_o = (qh + 1) * 1024 - qs_o
                P8t = ppool.tile([128, 2, 1024], FP8, tag="P",
                                 name=f"P{hl}_{qh}_{c}")
                for (kk, qs, qlen, pl) in ((ke, qs_e, qlen_e, 0),
                                           (ko, qs_o, qlen_o, 1)):
                    St = sps.tile([128, 1024], F32, tag="S", space="PSUM",
                                  name=f"St{hl}_{qh}_{kk}")
                    emit_scores(St, kk, qs, qlen, KT, QT)
                    nc.scalar.activation(P8t[:, pl, 0:qlen], St[:, 0:qlen],
                                         EXP, scale=0.125)
                relP = relPs[c]
                if sliver:
                    a = relP - 128
                    nc.tensor.matmul(
                        outp[a // 512][:, a % 512:a % 512 + 128],
                        lhsT=Vc[:, ke, 0:65],
                        rhs=P8t[:, 0, 0:128],
                        start=(c == 0), stop=False,
                    )
                for (a, b) in _pieces(relP, relP + qlen_o):
                    rhs = bass.AP(
                        tensor=P8t.tensor,
                        offset=P8t.offset + sliver + (a - relP),
                        ap=[list(P8t.ap[0]), [1024 - sliver, 2], [1, b - a]],
                    )
                    nc.tensor.matmul(
                        outp[a // 512][:, a % 512:a % 512 + b - a],
                        lhsT=Vc[:, ke:ke + 2, 0:65],
                        rhs=rhs,
                        start=(c == 0), stop=(c == bank_last[a // 512]),
                        perf_mode=DR,
                    )

        def emit_attention_bf16(hl, qh, outp, KT, QT, Vc):
            # Key chunks whose query lengths sum to 1024 share one St tile
            # and ONE exp: the causal staircase pairs up exactly (896+128,
            # 768+256, 640+384, ...), cutting ACTIVATE count from 24 to 18
            # per head (~300ns fixed overhead each, and fewer exp-latency
            # ping-pong points for the PE).
            items = []
            for kc in range(8 * (qh + 1)):
                qstart = max(kc * 128, qh * 1024)
                items.append((kc, qstart, (qh + 1) * 1024 - qstart))
            full = [[it] for it in items if it[2] >= 1024]
            rest = sorted((it for it in items if it[2] < 1024),
                          key=lambda it: -it[2])
            groups = list(full)
            i, j = 0, len(rest) - 1
            while i <= j:
                if i < j and rest[i][2] + rest[j][2] <= 1024:
                    groups.append([rest[i], rest[j]])
                    i, j = i + 1, j - 1
                else:
                    groups.append([rest[i]])
                    i += 1
            groups.sort(key=lambda g: min(it[0] for it in g))
            # per-PSUM-bank last writer under the actual emission order
            bank_last = {}
            for g in groups:
                for (kc, qstart, qlen) in g:
                    rel = qstart - qh * 1024
                    for (a, b) in _pieces(rel, rel + qlen):
                        bank_last[a // 512] = kc
            for g in groups:
                St = sps.tile([128, 1024], F32, tag="S", space="PSUM",
                              name=f"St{hl}_{qh}_{g[0][0]}")
                off, offs = 0, []
                for (kc, qstart, qlen) in g:
                    for (a, b) in _pieces(off, off + qlen):
                        nc.tensor.matmul(
                            St[:, a:b],
                            lhsT=KT[:, kc * 128:(kc + 1) * 128],
                            rhs=QT[:, qstart + a - off: qstart + b - off],
                            start=True, stop=True,
                        )
                    offs.append(off)
                    off += qlen
                P = ppool.tile([128, 1024], DT, tag="P",
                               name=f"P{hl}_{qh}_{g[0][0]}")
                nc.scalar.activation(P[:, 0:off], St[:, 0:off], EXP, scale=0.125)
                for (kc, qstart, qlen), o in zip(g, offs):
                    if kc * 128 == qstart:
                        nc.vector.tensor_mul(
                            P[:, o:o + 128], P[:, o:o + 128], mask01)
                    rel = qstart - qh * 1024
                    for (a, b) in _pieces(rel, rel + qlen):
                        nc.tensor.matmul(
                            outp[a // 512][:, a % 512:a % 512 + b - a],
                            lhsT=Vc[:, kc, :],
                            rhs=P[:, o + a - rel: o + b - rel],
                            start=(kc == 0), stop=(kc == bank_last[a // 512]),
                        )

        # ---- softmax-denominator normalization chains.
        # Each (hl, qh) produces a chain: den row PSUM->DRAM, partition-spread
        # reload, DVE reciprocal, DRAM respread, broadcast reload, then the
        # con write (on GpSimd, off the DVE queue). The chain is emitted in
        # three slices across later flush points so no in-order engine queue
        # ever waits on a DMA round-trip hop in flight.
        chains = []

        def flush_chains():
            for ch in list(chains):
                ch.pop(0)()
                if not ch:
                    chains.remove(ch)

        def emit_attention(hl, qh):
            QT, KT, VT, Vc = preps[hl]
            con = conA if hl < 2 else conB
            r0 = 64 * (hl % 2)
            outpt = ops.tile([65, 1024], F32, tag="outp", space="PSUM",
                             name=f"outp{hl}_{qh}")
            outp = [outpt[:, 0:512], outpt[:, 512:1024]]
            if pv_fp8:
                emit_attention_fp8(hl, qh, outp, KT, QT, Vc)
            else:
                emit_attention_bf16(hl, qh, outp, KT, QT, Vc)
            stg = small.tile([65, 1024], F32, tag="stg", name=f"stg{hl}_{qh}")
            nc.vector.tensor_copy(stg, outpt)
            # denominator row, partition-spread via DRAM (a (1,1024)
            # single-lane DVE reciprocal costs 6.5us; spread across 128
            # partitions it is ~60ns).
            d_dram = dpool.tile([1, 1024], F32, tag="d_dram", name=f"dd{hl}_{qh}")
            nc.sync.dma_start(out=d_dram, in_=stg[64:65, :])
            spread = small.tile([128, 8], F32, tag="spread", name=f"sp{hl}_{qh}")
            nc.sync.dma_start(
                out=spread,
                in_=d_dram.rearrange("a (p i) -> p a i", p=128)[:, 0, :],
            )
            box = {}

            def s2():
                rspread = small.tile([128, 8], F32, tag="rspread",
                                     name=f"rs{hl}_{qh}")
                nc.vector.reciprocal(rspread, spread)
                r_dram = dpool.tile([1, 1024], F32, tag="r_dram",
                                    name=f"rd{hl}_{qh}")
                nc.sync.dma_start(
                    out=r_dram.rearrange("a (p i) -> p a i", p=128)[:, 0, :],
                    in_=rspread,
                )
                rec64 = small.tile([64, 1024], F32, tag="rec64",
                                   name=f"r64{hl}_{qh}")
                rec_bcast = bass.AP(
                    tensor=r_dram.tensor, offset=r_dram.offset,
                    ap=[[0, 64]] + [list(d) for d in r_dram.ap[1:]],
                )
                nc.sync.dma_start(out=rec64, in_=rec_bcast)
                box["rec64"] = rec64

            def s3():
                # stg/rec64 columns are chunk-major permuted; iterate in true
                # pseudo-position order (innermost 16 contiguous) so con
                # comes out unpermuted for the output projection.
                perm = "d (qc jb sq) -> d qc sq jb"
                nc.gpsimd.tensor_mul(
                    con[r0:r0 + 64, qh * 1024:(qh + 1) * 1024].rearrange(
                        "d (qc sq jb) -> d qc sq jb", qc=8, sq=8),
                    stg[0:64, :].rearrange(perm, qc=8, jb=16),
                    box["rec64"].rearrange(perm, qc=8, jb=16),
                )

            chains.append([s2, s3])

        for hl in range(HPC):
            emit_prep(hl)
            for qh in range(2):
                emit_attention(hl, qh)
                flush_chains()
            preps.pop(hl)
        while chains:
            flush_chains()
        attn_ctx.close()
        oproj = ctx.enter_context(tc.tile_pool(name="oproj", bufs=5, space="PSUM"))

        # ---- output projection: partialT[j, s] = woutT_s.T @ [conA; conB]
        # PSUM->SBUF staging copies alternate ACT/DVE so neither engine's
        # ~0.5us-per-tile copy serializes the 32-tile drain.
        # sb-outer: the sb=0,1 tiles only need the qh=0 halves of con, which
        # are ready before the last qh=1 normalization chains drain.
        for i, (sb, jc) in enumerate(
                (sb, jc) for sb in range(4) for jc in range(8)):
            ps = oproj.tile([128, 512], F32, tag="op")
            nc.tensor.matmul(
                ps, lhsT=wout_sb[:, 0, jc * 128:(jc + 1) * 128],
                rhs=conA[:, sb * 512:(sb + 1) * 512],
                start=True, stop=False,
            )
            nc.tensor.matmul(
                ps, lhsT=wout_sb[:, 1, jc * 128:(jc + 1) * 128],
                rhs=conB[:, sb * 512:(sb + 1) * 512],
                start=False, stop=True,
            )
            st = stpool.tile([128, 512], BF16, tag="st")
            if i % 2 == 0:
                nc.scalar.copy(st, ps)
            else:
                nc.vector.tensor_copy(st, ps)
            r = (sb * 8 + jc) * 128
            nc.sync.dma_start(out=outT[r:r + 128, :], in_=st)
    nc.compile()
    return nc


def make_in_maps(x, Wqkv, bqkv, Wout, mm_dt=BF16, pv_fp8=False):
    np_dt = mybir.dt.np(mm_dt)
    x = np.asarray(x, np.float32)
    xT = np.ascontiguousarray(x.transpose(0, 2, 1)).astype(np_dt)  # (2,1024,2048)
    WqkvT = np.asarray(Wqkv, np.float32).T.astype(np_dt)
    WoutT = np.asarray(Wout, np.float32).T.astype(np_dt)
    bq = np.asarray(bqkv, np.float32).reshape(1, 3 * E)
    # wq[(t*8+ec)*128+p, c] = WqkvT[ec*128+p, t*1024+c]
    wqh = np.ascontiguousarray(
        WqkvT.reshape(8, 128, 3, 1024).transpose(2, 0, 1, 3).reshape(3072, 1024))
    # mask for the diagonal 128-chunk: key partitions AND query columns are
    # both in within-chunk scatter order (idx = 8*jb + s8 -> true 16*s8 + jb).
    p = np.arange(128)
    pos = 16 * (p % 8) + p // 8
    allowed = pos[:, None] <= pos[None, :]
    if pv_fp8:
        maskp = np.where(allowed, 0.0, -1e9).astype(np.float32)
    else:
        maskp = allowed.astype(np_dt)
    in_maps = []
    for c in range(8):
        b, qd = divmod(c, 4)
        xc = xT[b][:, qd * SL:(qd + 1) * SL]  # [1024, 512]
        # xq[hl*128+p, ec*128+s] = xc[ec*128+p, hl*128+s]
        xqh = np.ascontiguousarray(
            xc.reshape(8, 128, 4, 128).transpose(2, 1, 0, 3).reshape(512, 1024))
        wo = WoutT[qd * 256:(qd + 1) * 256, :]  # [256, 1024]
        woq = np.ascontiguousarray(
            wo.reshape(2, 128, E).transpose(1, 0, 2).reshape(128, 2 * E))
        in_maps.append({
            "xq": xqh,
            "wq": wqh,
            "bqkv": bq,
            "woutq": woq,
            "maskp": maskp,
        })
    return in_maps


_NC_CACHE = {}


def get_program(mm_dt=BF16, pv_fp8=False):
    key = (str(mm_dt), pv_fp8)
    if key not in _NC_CACHE:
        _NC_CACHE[key] = build_program(mm_dt, pv_fp8)
    return _NC_CACHE[key]


def assemble(results, bout):
    bout = np.asarray(bout, np.float32)
    out = np.zeros((B, S, E), np.float32)
    for c in range(8):
        b = c // 4
        # tile-major [sb, jc, 128, 512] -> [E, S] -> transpose to [S, E]
        pt = results[c]["partialT"].reshape(4, 8, 128, 512).astype(np.float32)
        out[b] += pt.transpose(0, 3, 1, 2).reshape(S, E)
    out += bout
    return out


def kernel(x, Wqkv, bqkv, Wout, bout, mm_dt=BF16, pv_fp8=False, trace=False):
    nc = get_program(mm_dt, pv_fp8)
    in_maps = make_in_maps(x, Wqkv, bqkv, Wout, mm_dt, pv_fp8)
    res = run_bass_kernel_spmd(nc, in_maps, list(range(8)), trace=trace)
    out = assemble(res.results, bout)
    if trace:
        kernel.last_result = res
    return out



# revision 79
# speedup vs baseline: 1.0614x; 1.0094x over previous
"""Trainium2 Bass kernel for nn_MultiHeadAttention_67018669687091.

Problem: MHA with B=2, S=2048, E=1024, H=16, D=64, causal, fp32.
The reference reshapes (B,S,E)->(B,H,S,D) WITHOUT transpose, so head h of
batch b is the contiguous 128-row x-block rows [h*128,(h+1)*128) viewed as a
(2048, 64) pseudo-sequence: position 16*s+j <- (row s, channel 64j+d).

Sharding: 8 cores; core c owns batch b=c//4 and head-quad qd=c%4 (4 heads).
Each core computes the qkv projection for its 4 blocks, per-head causal
attention in the transposed domain (scores with key on partitions, softmax
along the free dim via an augmented ones-row of V and late normalization),
and a row-parallel slice of the output projection. Host sums the 4 partials
per batch and adds bout.

Performance notes (per-core, PE-column-bound; the PE clock is power-governed
so matmul column count is the controlling resource):
 - QT/KT/VT are stored chunk-major permuted (col = 128*kc + 8*jb + s8) so
   the qkv->per-head scatter copies are contiguous-innermost on the DVE
   (4.6us -> ~1.2us per copy); all causal chunk slicing is unchanged, the
   host-built mask handles the within-chunk order, and the permutation is
   undone for free in the con write.
 - Inputs are host-prepacked tile-contiguous; x/bias issue from the Scalar
   HWDGE queue in parallel with weights on Sync (dma_start issue time, not
   bandwidth, gates the prologue). The partial output is written tile-major.
 - Softmax denominator reciprocal: partition-spread via a DRAM round trip,
   emitted in deferred slices across later flush points so no in-order
   engine queue waits on a DMA hop; the con multiply runs on GpSimd.
 - Attention-phase PSUM pools close before the output projection so it gets
   a 5-deep pool (with 2 bufs each tile's matmuls serialize behind the
   previous tile's staging copy).
 - fp8 DoubleRow PV (pv_fp8=True) works but is a net loss: the power
   governor tracks MAC-rate, which DoubleRow does not reduce, and exp
   overflows e4m3 without per-head max subtraction (NaN). Kept for
   reference, off by default.

Matmul operand dtype is selectable: bfloat16 (fastest), float32r, float32.
"""
import numpy as np
from contextlib import ExitStack

import concourse.bass as bass
import concourse.bacc as bacc
import concourse.mybir as mybir
import concourse.tile as tile
from concourse.masks import make_identity
from concourse.bass_utils import run_bass_kernel_spmd

E = 1024
H = 16
D = 64
B = 2
S = 2048
HPC = 4          # heads per core
SL = HPC * 128   # x columns per core (512)

F32 = mybir.dt.float32
F32R = mybir.dt.float32r
BF16 = mybir.dt.bfloat16
FP8 = mybir.dt.float8e4
EXP = mybir.ActivationFunctionType.Exp
DR = mybir.MatmulPerfMode.DoubleRow


def _pieces(lo, hi, bank=512):
    """Split [lo, hi) at multiples of `bank` (PSUM bank boundaries)."""
    out = []
    while lo < hi:
        nxt = min(hi, (lo // bank + 1) * bank)
        out.append((lo, nxt))
        lo = nxt
    return out


def build_program(mm_dt=BF16, pv_fp8=False):
    """One SPMD program; per-core data comes via in_maps."""
    nc = bacc.Bacc("TRN2", target_bir_lowering=False)
    DT = mm_dt
    # dtype of the qkv tiles / transposes: must pair legally with identity
    QKVDT = DT if DT == BF16 else F32

    # Inputs host-prepacked so that every load is a single clean 2D DMA with
    # contiguous per-partition lines (cheap Sync-issue descriptors):
    #   xq[hl*128+p, ec*128+s]      = x[b].T[ec*128+p, hl*128+s]
    #   wq[(t*8+ec)*128+p, c]       = Wqkv.T[ec*128+p, t*1024+c]
    #   woutq[p, hf*E+j]            = Wout.T[qd*256+hf*128+p, j]
    xq = nc.dram_tensor("xq", [HPC * 128, 1024], DT, kind="ExternalInput").ap()
    wq = nc.dram_tensor("wq", [24 * 128, 1024], DT, kind="ExternalInput").ap()
    bqkv = nc.dram_tensor("bqkv", [1, 3 * E], F32, kind="ExternalInput").ap()
    woutq = nc.dram_tensor("woutq", [128, 2 * E], DT, kind="ExternalInput").ap()
    maskd = nc.dram_tensor(
        "maskp", [128, 128], F32 if pv_fp8 else BF16, kind="ExternalInput").ap()
    # tile-major output: row block (sb*8+jc) holds the [128, 512] tile for
    # out channels jc*128.. and positions sb*512.. — every DMA write lands
    # fully contiguous in DRAM; the host assembles the final layout.
    outT = nc.dram_tensor("partialT", [32 * 128, 512], BF16,
                          kind="ExternalOutput").ap()

    with tile.TileContext(nc) as tc, ExitStack() as ctx:
        singles = ctx.enter_context(tc.tile_pool(name="singles", bufs=1))
        wpool = ctx.enter_context(tc.tile_pool(name="wpool", bufs=2))
        qkvpool = ctx.enter_context(tc.tile_pool(name="qkvpool", bufs=1))
        headpool = ctx.enter_context(tc.tile_pool(name="headpool", bufs=2))
        ppool = ctx.enter_context(tc.tile_pool(name="ppool", bufs=4))
        cpool = ctx.enter_context(tc.tile_pool(name="cpool", bufs=1))
        stpool = ctx.enter_context(tc.tile_pool(name="stpool", bufs=4))
        small = ctx.enter_context(tc.tile_pool(name="small", bufs=3))
        dpool = ctx.enter_context(tc.tile_pool(name="dpool", bufs=2, space="DRAM"))
        mmps = ctx.enter_context(tc.tile_pool(name="mmps", bufs=2, space="PSUM"))
        # attention-phase PSUM pools live in their own scope: they close
        # before the output projection so their 6 banks can be reused for a
        # deeply-buffered out-proj pool (bufs=2 there serializes each tile's
        # matmuls behind the previous tile's staging copy).
        attn_ctx = ExitStack()
        sps = attn_ctx.enter_context(tc.tile_pool(name="sps", bufs=2, space="PSUM"))
        ops = attn_ctx.enter_context(tc.tile_pool(name="ops", bufs=1, space="PSUM"))

        ident = singles.tile([128, 128], QKVDT, tag="ident")
        make_identity(nc, ident)

        # Diagonal-chunk causal mask, host-computed for the permuted key/query
        # order (idx = 8*jb + s8 -> pos 16*s8 + jb). pv_fp8: additive 0/-1e9
        # f32 mask applied to scores pre-exp. bf16: multiplicative 0/1 on P.
        # (loaded after the projection emission — not startup-critical)
        mask01 = singles.tile([128, 128], F32 if pv_fp8 else BF16, tag="mask01")

        # Startup-critical loads only: x tiles and the qkv bias. wout and the
        # mask are deferred until after the projection loop — every dma_start
        # costs ~0.65us of serial issue time on the Sync queue, so front
        # issue-count is what sets when the first matmul can run.
        # x + bias issue from the (startup-idle) Scalar queue, weights from
        # Sync — the two HWDGE issuers run in parallel, halving the serial
        # issue time in front of the first projection matmul.
        xts = []
        for hl in range(HPC):
            xt = singles.tile([128, 8, 128], DT, tag=f"xt{hl}")
            nc.scalar.dma_start(out=xt, in_=xq[hl * 128:(hl + 1) * 128, :])
            xts.append(xt)

        bias_sb = singles.tile([128, 3 * E], F32, tag="bias")
        bq_bcast = bass.AP(
            tensor=bqkv.tensor, offset=bqkv.offset,
            ap=[[0, 128]] + [list(d) for d in bqkv.ap[1:]],
        )
        nc.scalar.dma_start(out=bias_sb, in_=bq_bcast)

        # ---- QKV projection: qkv[hl] = x_blk @ WqkvT + bqkv  (128, 3072)
        qkvs = [
            qkvpool.tile([128, 3 * E], QKVDT, tag=f"qkv{hl}", name=f"qkv{hl}")
            for hl in range(HPC)
        ]

        # ---- per-head attention prep machinery. qkv columns of tensor t
        # (q/k/v) are exactly proj nb-blocks (2t, 2t+1), so each tensor's
        # transposes + scatter copies are emitted right after its two proj
        # blocks: the DVE scatter stream for heads 0/1 runs during the
        # projection (where DVE is otherwise idle) instead of after it.
        conA = cpool.tile([128, S], DT, tag="conA")
        conB = cpool.tile([128, S], DT, tag="conB")
        preps = {}

        def head_tiles(hl):
            if hl not in preps:
                # QT/KT zero-padded to 128 partitions: K=128 full-array S
                # matmuls stream at 216ns (K=64 serializes LDWEIGHTS).
                QT = headpool.tile([128, S], DT, tag="QT", name=f"QT{hl}")
                KT = headpool.tile([128, S], DT, tag="KT", name=f"KT{hl}")
                VT = headpool.tile([65, S], QKVDT, tag="VT", name=f"VT{hl}")
                # pv_fp8: inner dim padded to 80 — DoubleRow LDWEIGHTS requires
                # the K-pair stride to be a multiple of 16.
                Vc = headpool.tile([128, 16, 80 if pv_fp8 else 65],
                                   FP8 if pv_fp8 else DT,
                                   tag="Vc", name=f"Vc{hl}")
                if hl < 2:
                    # pads/ones are written once per pool slot (bufs=2, slots
                    # alternate hl%2); heads 2/3 reuse them — scatters only
                    # touch rows 0:64, so the constant rows persist and the
                    # per-head memset (plus its WAR serialization in the prep
                    # chain) is skipped.
                    nc.gpsimd.memset(QT[64:128, :], 0.0)
                    nc.gpsimd.memset(KT[64:128, :], 0.0)
                    nc.gpsimd.memset(VT[64:65, :], 1.0)
                preps[hl] = [QT, KT, VT, Vc]
            return preps[hl]

        def emit_tensor_prep(hl, t):
            # QT/KT/VT hold the head's (d, pseudo-seq) slab chunk-major:
            # column 128*kc + 8*jb + s8, for pseudo-position 16*(8*kc+s8)+jb.
            # The permutation is local to each 128-chunk, so all causal
            # chunk/piece slicing is unchanged; the scatter copy below gets a
            # contiguous innermost run (fast on DVE) instead of a stride-16
            # element scatter (4.6us -> ~1.2us per copy). Scores/P/outp/stg
            # inherit the within-chunk column order; it is undone in the con
            # write, and the host-built mask01 accounts for it.
            dest = head_tiles(hl)[t]
            tpb = mmps.tile([128, 1024], QKVDT, tag="mm", space="PSUM",
                            name=f"tpb{hl}_{t}")
            for cc in range(8):
                nc.tensor.transpose(
                    tpb[:, cc * 128:(cc + 1) * 128],
                    qkvs[hl][:, t * 1024 + cc * 128: t * 1024 + (cc + 1) * 128],
                    ident,
                )
            t3 = tpb.rearrange("p (cc s) -> p cc s", cc=8)
            for jp in range(2):
                csrc = t3[64 * jp:64 * jp + 64, :, :].rearrange(
                    "d cc (kc sk) -> d cc kc sk", kc=16)
                dd = dest[0:64].rearrange(
                    "d (kc cc two sk) -> d two cc kc sk",
                    kc=16, cc=8, two=2)[:, jp, :, :, :]
                nc.vector.tensor_copy(dd, csrc)

        def emit_vc_prep(hl):
            # four chunk-transposes per PSUM tile, drained by ONE strided DVE
            # copy — 4 instead of 16 copies per head, and the PE/DVE
            # ping-pong through the 2 pool slots happens per group of four.
            QT, KT, VT, Vc = head_tiles(hl)
            for g in range(4):
                tp = mmps.tile([128, 1024], QKVDT, tag="mm", space="PSUM",
                               name=f"tpv{hl}_{g}")
                for i in range(4):
                    kc = 4 * g + i
                    nc.tensor.transpose(
                        tp[:, 256 * i:256 * i + 65],
                        VT[:, kc * 128:(kc + 1) * 128], ident[0:65, 0:65])
                nc.vector.tensor_copy(
                    Vc[:, 4 * g:4 * g + 4, 0:65],
                    tp.rearrange("p (i c) -> p i c", i=4)[:, :, 0:65])

        def emit_prep(hl):
            for t in range(3):
                emit_tensor_prep(hl, t)
            emit_vc_prep(hl)

        # ---- QKV projection. Weights come in as one [128, 1024] DMA per
        # (t, ec) covering both nb blocks of tensor t: halves the number of
        # Sync-issued dma_starts on the startup-critical path.
        for t in range(3):
            wts = []
            for ec in range(8):
                wtc = wpool.tile([128, 1024], DT, tag=f"wt{ec}", name=f"wt{t}_{ec}")
                r = (t * 8 + ec) * 128
                nc.sync.dma_start(out=wtc, in_=wq[r:r + 128, :])
                wts.append(wtc)
            for nb in (2 * t, 2 * t + 1):
                h = (nb % 2) * 512
                for hl in range(HPC):
                    ps = mmps.tile([128, 512], F32, tag="mm")
                    for ec in range(8):
                        nc.tensor.matmul(
                            ps, lhsT=xts[hl][:, ec, :], rhs=wts[ec][:, h:h + 512],
                            start=(ec == 0), stop=(ec == 7),
                        )
                    nc.vector.tensor_add(
                        qkvs[hl][:, nb * 512:(nb + 1) * 512], ps,
                        bias_sb[:, nb * 512:(nb + 1) * 512],
                    )

        # deferred non-critical input loads (used from attention onward)
        nc.sync.dma_start(out=mask01, in_=maskd)
        wout_sb = singles.tile([128, 2, E], DT, tag="wout")
        nc.sync.dma_start(out=wout_sb, in_=woutq.rearrange("p (hf j) -> p hf j", hf=2))

        def emit_scores(St, kc, qstart, qlen, KT, QT):
            for (a, b) in _pieces(0, qlen):
                nc.tensor.matmul(
                    St[:, a:b],
                    lhsT=KT[:, kc * 128:(kc + 1) * 128],
                    rhs=QT[:, qstart + a: qstart + b],
                    start=True, stop=True,
                )
            if kc * 128 == qstart and pv_fp8:
                # additive -1e9 mask on the diagonal chunk, pre-exp
                nc.vector.tensor_add(St[:, 0:128], St[:, 0:128], mask01)

        def emit_attention_fp8(hl, qh, outp, KT, QT, Vc):
            # Key chunks processed in pairs (2c, 2c+1); P stored as fp8 planes
            # and PV runs as K=256 DoubleRow matmuls (half the PE columns).
            # The even chunk's extra 128 queries (diagonal sliver) get a
            # separate plain-fp8 matmul.
            npairs = 4 * (qh + 1)
            relPs = [max(128 * (2 * c + 1) - 1024 * qh, 0) for c in range(npairs)]
            bank_last = {
                bk: max(c for c in range(npairs) if relPs[c] < 512 * (bk + 1))
                for bk in range(2)
            }
            for c in range(npairs):
                ke, ko = 2 * c, 2 * c + 1
                qs_e = max(ke * 128, qh * 1024)
                qs_o = max(ko * 128, qh * 1024)
                sliver = qs_o - qs_e  # 0 or 128
                qlen_e = (qh + 1) * 1024 - qs_e
                qlen_o = (qh + 1) * 1024 - qs_o
                P8t = ppool.tile([128, 2, 1024], FP8, tag="P",
                                 name=f"P{hl}_{qh}_{c}")
                for (kk, qs, qlen, pl) in ((ke, qs_e, qlen_e, 0),
                                           (ko, qs_o, qlen_o, 1)):
                    St = sps.tile([128, 1024], F32, tag="S", space="PSUM",
                                  name=f"St{hl}_{qh}_{kk}")
                    emit_scores(St, kk, qs, qlen, KT, QT)
                    nc.scalar.activation(P8t[:, pl, 0:qlen], St[:, 0:qlen],
                                         EXP, scale=0.125)
                relP = relPs[c]
                if sliver:
                    a = relP - 128
                    nc.tensor.matmul(
                        outp[a // 512][:, a % 512:a % 512 + 128],
                        lhsT=Vc[:, ke, 0:65],
                        rhs=P8t[:, 0, 0:128],
                        start=(c == 0), stop=False,
                    )
                for (a, b) in _pieces(relP, relP + qlen_o):
                    rhs = bass.AP(
                        tensor=P8t.tensor,
                        offset=P8t.offset + sliver + (a - relP),
                        ap=[list(P8t.ap[0]), [1024 - sliver, 2], [1, b - a]],
                    )
                    nc.tensor.matmul(
                        outp[a // 512][:, a % 512:a % 512 + b - a],
                        lhsT=Vc[:, ke:ke + 2, 0:65],
                        rhs=rhs,
                        start=(c == 0), stop=(c == bank_last[a // 512]),
                        perf_mode=DR,
                    )

        def emit_attention_bf16(hl, qh, outp, KT, QT, Vc):
            # Key chunks whose query lengths sum to 1024 share one St tile
            # and ONE exp: the causal staircase pairs up exactly (896+128,
            # 768+256, 640+384, ...), cutting ACTIVATE count from 24 to 18
            # per head (~300ns fixed overhead each, and fewer exp-latency
            # ping-pong points for the PE).
            items = []
            for kc in range(8 * (qh + 1)):
                qstart = max(kc * 128, qh * 1024)
                items.append((kc, qstart, (qh + 1) * 1024 - qstart))
            full = [[it] for it in items if it[2] >= 1024]
            rest = sorted((it for it in items if it[2] < 1024),
                          key=lambda it: -it[2])
            groups = list(full)
            i, j = 0, len(rest) - 1
            while i <= j:
                if i < j and rest[i][2] + rest[j][2] <= 1024:
                    groups.append([rest[i], rest[j]])
                    i, j = i + 1, j - 1
                else:
                    groups.append([rest[i]])
                    i += 1
            groups.sort(key=lambda g: min(it[0] for it in g))
            # per-PSUM-bank last writer under the actual emission order
            bank_last = {}
            for g in groups:
                for (kc, qstart, qlen) in g:
                    rel = qstart - qh * 1024
                    for (a, b) in _pieces(rel, rel + qlen):
                        bank_last[a // 512] = kc
            for g in groups:
                St = sps.tile([128, 1024], F32, tag="S", space="PSUM",
                              name=f"St{hl}_{qh}_{g[0][0]}")
                off, offs = 0, []
                for (kc, qstart, qlen) in g:
                    for (a, b) in _pieces(off, off + qlen):
                        nc.tensor.matmul(
                            St[:, a:b],
                            lhsT=KT[:, kc * 128:(kc + 1) * 128],
                            rhs=QT[:, qstart + a - off: qstart + b - off],
                            start=True, stop=True,
                        )
                    offs.append(off)
                    off += qlen
                P = ppool.tile([128, 1024], DT, tag="P",
                               name=f"P{hl}_{qh}_{g[0][0]}")
                nc.scalar.activation(P[:, 0:off], St[:, 0:off], EXP, scale=0.125)
                for (kc, qstart, qlen), o in zip(g, offs):
                    if kc * 128 == qstart:
                        nc.vector.tensor_mul(
                            P[:, o:o + 128], P[:, o:o + 128], mask01)
                    rel = qstart - qh * 1024
                    for (a, b) in _pieces(rel, rel + qlen):
                        nc.tensor.matmul(
                            outp[a // 512][:, a % 512:a % 512 + b - a],
                            lhsT=Vc[:, kc, :],
                            rhs=P[:, o + a - rel: o + b - rel],
                            start=(kc == 0), stop=(kc == bank_last[a // 512]),
                        )

        # ---- softmax-denominator normalization chains.
        # Each (hl, qh) produces a chain: den row PSUM->DRAM, partition-spread
        # reload, DVE reciprocal, DRAM respread, broadcast reload, then the
        # con write (on GpSimd, off the DVE queue). The chain is emitted in
        # three slices across later flush points so no in-order engine queue
        # ever waits on a DMA round-trip hop in flight.
        chains = []

        def flush_chains():
            for ch in list(chains):
                ch.pop(0)()
                if not ch:
                    chains.remove(ch)

        def emit_attention(hl, qh):
            QT, KT, VT, Vc = preps[hl]
            con = conA if hl < 2 else conB
            r0 = 64 * (hl % 2)
            outpt = ops.tile([65, 1024], F32, tag="outp", space="PSUM",
                             name=f"outp{hl}_{qh}")
            outp = [outpt[:, 0:512], outpt[:, 512:1024]]
            if pv_fp8:
                emit_attention_fp8(hl, qh, outp, KT, QT, Vc)
            else:
                emit_attention_bf16(hl, qh, outp, KT, QT, Vc)
            stg = small.tile([65, 1024], F32, tag="stg", name=f"stg{hl}_{qh}")
            nc.vector.tensor_copy(stg, outpt)
            # denominator row, partition-spread via DRAM (a (1,1024)
            # single-lane DVE reciprocal costs 6.5us; spread across 128
            # partitions it is ~60ns).
            d_dram = dpool.tile([1, 1024], F32, tag="d_dram", name=f"dd{hl}_{qh}")
            nc.sync.dma_start(out=d_dram, in_=stg[64:65, :])
            spread = small.tile([128, 8], F32, tag="spread", name=f"sp{hl}_{qh}")
            nc.sync.dma_start(
                out=spread,
                in_=d_dram.rearrange("a (p i) -> p a i", p=128)[:, 0, :],
            )
            box = {}

            def s2():
                rspread = small.tile([128, 8], F32, tag="rspread",
                                     name=f"rs{hl}_{qh}")
                nc.vector.reciprocal(rspread, spread)
                r_dram = dpool.tile([1, 1024], F32, tag="r_dram",
                                    name=f"rd{hl}_{qh}")
                nc.sync.dma_start(
                    out=r_dram.rearrange("a (p i) -> p a i", p=128)[:, 0, :],
                    in_=rspread,
                )
                rec64 = small.tile([64, 1024], F32, tag="rec64",
                                   name=f"r64{hl}_{qh}")
                rec_bcast = bass.AP(
                    tensor=r_dram.tensor, offset=r_dram.offset,
                    ap=[[0, 64]] + [list(d) for d in r_dram.ap[1:]],
                )
                nc.sync.dma_start(out=rec64, in_=rec_bcast)
                box["rec64"] = rec64

            def s3():
                # stg/rec64 columns are chunk-major permuted; iterate in true
                # pseudo-position order (innermost 16 contiguous) so con
                # comes out unpermuted for the output projection.
                perm = "d (qc jb sq) -> d qc sq jb"
                nc.gpsimd.tensor_mul(
                    con[r0:r0 + 64, qh * 1024:(qh + 1) * 1024].rearrange(
                        "d (qc sq jb) -> d qc sq jb", qc=8, sq=8),
                    stg[0:64, :].rearrange(perm, qc=8, jb=16),
                    box["rec64"].rearrange(perm, qc=8, jb=16),
                )

            chains.append([s2, s3])

        for hl in range(HPC):
            emit_prep(hl)
            for qh in range(2):
                emit_attention(hl, qh)
                flush_chains()
            preps.pop(hl)
        while chains:
            flush_chains()
        attn_ctx.close()
        oproj = ctx.enter_context(tc.tile_pool(name="oproj", bufs=6, space="PSUM"))

        # ---- output projection: partialT[j, s] = woutT_s.T @ [conA; conB]
        # PSUM->SBUF staging copies alternate ACT/DVE so neither engine's
        # ~0.5us-per-tile copy serializes the 32-tile drain.
        # sb-outer: the sb=0,1 tiles only need the qh=0 halves of con, which
        # are ready before the last qh=1 normalization chains drain.
        for i, (sb, jc) in enumerate(
                (sb, jc) for sb in range(4) for jc in range(8)):
            ps = oproj.tile([128, 512], F32, tag="op")
            nc.tensor.matmul(
                ps, lhsT=wout_sb[:, 0, jc * 128:(jc + 1) * 128],
                rhs=conA[:, sb * 512:(sb + 1) * 512],
                start=True, stop=False,
            )
            nc.tensor.matmul(
                ps, lhsT=wout_sb[:, 1, jc * 128:(jc + 1) * 128],
                rhs=conB[:, sb * 512:(sb + 1) * 512],
                start=False, stop=True,
            )
            st = stpool.tile([128, 512], BF16, tag="st")
            if i % 2 == 0:
                nc.scalar.copy(st, ps)
            else:
                nc.vector.tensor_copy(st, ps)
            r = (sb * 8 + jc) * 128
            nc.sync.dma_start(out=outT[r:r + 128, :], in_=st)
    nc.compile()
    return nc


def make_in_maps(x, Wqkv, bqkv, Wout, mm_dt=BF16, pv_fp8=False):
    np_dt = mybir.dt.np(mm_dt)
    x = np.asarray(x, np.float32)
    xT = np.ascontiguousarray(x.transpose(0, 2, 1)).astype(np_dt)  # (2,1024,2048)
    WqkvT = np.asarray(Wqkv, np.float32).T.astype(np_dt)
    WoutT = np.asarray(Wout, np.float32).T.astype(np_dt)
    bq = np.asarray(bqkv, np.float32).reshape(1, 3 * E)
    # wq[(t*8+ec)*128+p, c] = WqkvT[ec*128+p, t*1024+c]
    wqh = np.ascontiguousarray(
        WqkvT.reshape(8, 128, 3, 1024).transpose(2, 0, 1, 3).reshape(3072, 1024))
    # mask for the diagonal 128-chunk: key partitions AND query columns are
    # both in within-chunk scatter order (idx = 8*jb + s8 -> true 16*s8 + jb).
    p = np.arange(128)
    pos = 16 * (p % 8) + p // 8
    allowed = pos[:, None] <= pos[None, :]
    if pv_fp8:
        maskp = np.where(allowed, 0.0, -1e9).astype(np.float32)
    else:
        maskp = allowed.astype(np_dt)
    in_maps = []
    for c in range(8):
        b, qd = divmod(c, 4)
        xc = xT[b][:, qd * SL:(qd + 1) * SL]  # [1024, 512]
        # xq[hl*128+p, ec*128+s] = xc[ec*128+p, hl*128+s]
        xqh = np.ascontiguousarray(
            xc.reshape(8, 128, 4, 128).transpose(2, 1, 0, 3).reshape(512, 1024))
        wo = WoutT[qd * 256:(qd + 1) * 256, :]  # [256, 1024]
        woq = np.ascontiguousarray(
            wo.reshape(2, 128, E).transpose(1, 0, 2).reshape(128, 2 * E))
        in_maps.append({
            "xq": xqh,
            "wq": wqh,
            "bqkv": bq,
            "woutq": woq,
            "maskp": maskp,
        })
    return in_maps


_NC_CACHE = {}


def get_program(mm_dt=BF16, pv_fp8=False):
    key = (str(mm_dt), pv_fp8)
    if key not in _NC_CACHE:
        _NC_CACHE[key] = build_program(mm_dt, pv_fp8)
    return _NC_CACHE[key]


def assemble(results, bout):
    bout = np.asarray(bout, np.float32)
    out = np.zeros((B, S, E), np.float32)
    for c in range(8):
        b = c // 4
        # tile-major [sb, jc, 128, 512] -> [E, S] -> transpose to [S, E]
        pt = results[c]["partialT"].reshape(4, 8, 128, 512).astype(np.float32)
        out[b] += pt.transpose(0, 3, 1, 2).reshape(S, E)
    out += bout
    return out


def kernel(x, Wqkv, bqkv, Wout, bout, mm_dt=BF16, pv_fp8=False, trace=False):
    nc = get_program(mm_dt, pv_fp8)
    in_maps = make_in_maps(x, Wqkv, bqkv, Wout, mm_dt, pv_fp8)
    res = run_bass_kernel_spmd(nc, in_maps, list(range(8)), trace=trace)
    out = assemble(res.results, bout)
    if trace:
        kernel.last_result = res
    return out

